# revision 1
# baseline (speedup 1.0000x reference)
"""Multi-head attention (B=4, Q=K=2048, D=512, H=8) on 8 TRN2 NeuronCores.

Sharding: data-parallel over batch across core pairs (4 batches x 2 cores),
tensor-parallel over heads within each pair (each core owns 4 of the 8 heads:
column-sharded W_q/W_k/W_v, row-sharded W_o).  Each core emits a partial
output projection for its batch; the host sums the two partials per batch.

Device-side layout choices:
  * All activations live transposed ([feature, seq]) so every matmul contracts
    over the partition dim with no on-chip transposes.
  * Scores are computed transposed (S_T[k, q] = K_h @ Q_h^T) so the valid-len
    key padding mask is a per-partition bias on the ACT exp instruction, and
    softmax needs no max-subtraction pass (scores are O(1) here; exp of the
    -1e6 masked entries underflows to exactly 0, matching the reference).
  * A ones-column interleaved into V makes the attnV matmul emit the softmax
    denominator for free (output row 64 of each head's [65, q] PSUM tile).
  * The key dim is truncated to max(valid_lens) rounded up to 128: dropped
    keys all have softmax weight exactly 0, so this is exact.
  * The whole matmul pipeline runs in bf16 with fp32 PSUM accumulation
    (plain fp32 matmuls are 4x slower on the PE and fp32 weight loads can't
    use fast-weight-load); softmax/normalization stay fp32.  Host converts
    inputs to bf16, which also halves the input DMA traffic.
"""

import ml_dtypes
import numpy as np

import concourse.bacc as bacc
import concourse.bass as bass
import concourse.mybir as mybir
from concourse import tile
from concourse.bass_utils import run_bass_kernel_spmd

F32 = mybir.dt.float32
F32R = mybir.dt.float32r
BF16 = mybir.dt.bfloat16

B, Q, KSEQ, D, H = 4, 2048, 2048, 512, 8
DH = D // H          # 64  head dim
HL = H // 2          # 4   local heads per core
DL = HL * DH         # 256 local features per core
NEG = -1.0e6
N_CORES = 8


def build_nc(KT: int):
    """Build the single-core SPMD program for a key length of KT (mult of 128)."""
    assert KT % 128 == 0 and 128 <= KT <= KSEQ
    KTC = KT // 128                      # number of 128-wide key chunks
    NQ = Q // 512                        # 4 q-chunks of 512
    KCH = [(s, min(512, KT - s)) for s in range(0, KT, 512)]
    EXP = mybir.ActivationFunctionType.Exp

    nc = bacc.Bacc("TRN2", target_bir_lowering=False, debug=False,
                   num_devices=N_CORES)

    def din(name, shape, dt=BF16):
        return nc.dram_tensor(name, shape, dt, kind="ExternalInput").ap()

    xq_d = din("xq_t", [D, Q])
    xk_d = din("xk_t", [D, KT])
    xv_d = din("xv_t", [D, KT])
    wq_d = din("wq_t", [D, DL])
    wk_d = din("wk_t", [D, DL])
    wv_d = din("wv_t", [D, DL])
    wo_d = din("wo_t", [DL, D])
    mask_d = din("mask", [128, KTC], F32)
    y_d = nc.dram_tensor("y_t", [D, Q], F32, kind="ExternalOutput").ap()

    with tile.TileContext(nc) as tc:
        with (
            # bf16 rounding on PSUM->SBUF copies is deliberate (see docstring)
            nc.allow_low_precision(reason="bf16 matmul operands"),
            tc.tile_pool(name="persist", bufs=1) as pp,
            tc.tile_pool(name="xpool", bufs=8) as xp,
            tc.tile_pool(name="cbuf", bufs=1) as cb,
            # 8 PSUM banks: psA 2x[128,512] (projections / broadcast / output
            # projection), psS 2x[128,1024] score tiles, psO 2x[65,512]
            # attention accumulators.
            tc.tile_pool(name="psA", bufs=2, space=bass.MemorySpace.PSUM) as psA,
            tc.tile_pool(name="psS", bufs=2, space=bass.MemorySpace.PSUM) as psS,
            tc.tile_pool(name="psO", bufs=1, space=bass.MemorySpace.PSUM) as psO,
        ):
            # ---- constants / weights / mask ----
            wq = [pp.tile([128, DL], BF16, tag=f"wq{i}", name=f"wq{i}") for i in range(4)]
            wk = [pp.tile([128, DL], BF16, tag=f"wk{i}", name=f"wk{i}") for i in range(4)]
            wv = [pp.tile([128, DL], BF16, tag=f"wv{i}", name=f"wv{i}") for i in range(4)]
            wo = [pp.tile([128, D], BF16, tag=f"wo{i}", name=f"wo{i}") for i in range(2)]
            for i in range(4):
                nc.sync.dma_start(wq[i][:], wq_d[i * 128:(i + 1) * 128, :])
            mask_sb = pp.tile([128, KTC], F32, tag="mask", name="mask_sb")
            nc.sync.dma_start(mask_sb[:], mask_d[:])
            onescr = pp.tile([128, DH], F32, tag="onescr", name="onescr")
            nc.vector.memset(onescr[:], 1.0)
            # row 64 is the broadcast-matmul lhsT (must share base partition
            # with the denominator row it multiplies against)
            ones_sb = pp.tile([65, DH], F32R, tag="ones", name="ones_sb")
            nc.vector.tensor_copy(ones_sb[64:65, :], onescr[64:65, :])

            # ---- Q projection:  q_t[o, q] = (Wq_loc/8) @ x_q  (transposed) ----
            # first input is DMA'd in 512-column chunks so the first matmul
            # group can start as early as possible
            xq = [xp.tile([128, Q], BF16, tag="x", name=f"x{i}") for i in range(4)]
            xk = [xp.tile([128, Q], BF16, tag="x", name=f"xk{i}") for i in range(4)]
            for i in range(4):
                nc.sync.dma_start(xq[i][:, 0:512],
                                  xq_d[i * 128:(i + 1) * 128, 0:512])
            # K-side loads issue ahead of the remaining q transfers so the
            # K projection isn't stuck ~19us deep in the sync DMA queue
            for i in range(4):
                nc.sync.dma_start(wk[i][:], wk_d[i * 128:(i + 1) * 128, :])
            for i in range(4):
                nc.sync.dma_start(xk[i][:, :KT], xk_d[i * 128:(i + 1) * 128, :])
            for i in range(4):
                nc.sync.dma_start(xq[i][:, 512:Q],
                                  xq_d[i * 128:(i + 1) * 128, 512:Q])
            for i in range(4):
                nc.sync.dma_start(wv[i][:], wv_d[i * 128:(i + 1) * 128, :])
            for i in range(2):
                nc.sync.dma_start(wo[i][:], wo_d[i * 128:(i + 1) * 128, :])
            q_t = [pp.tile([128, Q], BF16, tag=f"q_t{i}", name=f"q_t{i}") for i in range(2)]

            def qproj(ot, qs):
                ps = psA.tile([128, 512], F32, tag="proj", name="ps")
                for ic in range(4):
                    nc.tensor.matmul(
                        ps[:],
                        (wq[ic][:, ot * 128:(ot + 1) * 128]),
                        (xq[ic][:, qs * 512:(qs + 1) * 512]),
                        start=(ic == 0), stop=(ic == 3))
                nc.vector.tensor_copy(q_t[ot][:, qs * 512:(qs + 1) * 512], ps[:])

            for qs in range(NQ):
                qproj(0, qs)

            # ---- K projection:  k_t[o, k] ----
            k_t = [pp.tile([128, KT], BF16, tag=f"k_t{i}", name=f"k_t{i}") for i in range(2)]

            def kproj(ot, s, w):
                ps = psA.tile([128, 512], F32, tag="proj", name="ps")
                for ic in range(4):
                    nc.tensor.matmul(
                        ps[:, :w],
                        (wk[ic][:, ot * 128:(ot + 1) * 128]),
                        (xk[ic][:, s:s + w]),
                        start=(ic == 0), stop=(ic == 3))
                nc.vector.tensor_copy(k_t[ot][:, s:s + w], ps[:, :w])

            for (s, w) in KCH:
                kproj(0, s, w)
            for qs in range(NQ):
                qproj(1, qs)
            for (s, w) in KCH:
                kproj(1, s, w)

            # ---- V projection (emitted lazily, interleaved into the first
            # attention panel so the PE stream has no separate V phase and the
            # ACT engine starts on exps ~35us earlier).  Natural layout
            # v[k, o], heads interleaved with a ones column: per key-chunk
            # tile [128, 4*65], col h*65+64 == 1 (gives the softmax
            # denominator for free in the attnV matmul).
            xv = [xp.tile([128, Q], BF16, tag="x", name=f"x{i}") for i in range(4)]
            for i in range(4):
                nc.sync.dma_start(xv[i][:, :KT], xv_d[i * 128:(i + 1) * 128, :])
            v_sb = [pp.tile([128, HL * 65], BF16, tag=f"v{kt}", name=f"v{kt}") for kt in range(KTC)]

            def vproj(kt):
                ps = psA.tile([128, 512], F32, tag="proj", name="ps")
                for ic in range(4):
                    nc.tensor.matmul(
                        ps[:, :DL],
                        (xv[ic][:, kt * 128:(kt + 1) * 128]),
                        (wv[ic][:]),
                        start=(ic == 0), stop=(ic == 3))
                nc.vector.tensor_copy(v_sb[kt][:, 64::65], onescr[:, 0:HL])
                for h in range(HL):
                    nc.vector.tensor_copy(
                        v_sb[kt][:, h * 65:h * 65 + 64],
                        ps[:, h * 64:(h + 1) * 64])

            # ---- attention ----
            # q is processed in 1024-wide panels: two 512-wide scores matmuls
            # share one [128, 1024] PSUM tile so a single ACT exp covers both
            # (the mask bias is per-partition = per-key, constant across q).
            # The attnV matmuls run one k-chunk BEHIND the scores (software
            # pipeline) so the PE never stalls waiting for the exp that feeds
            # them -- a gap-free PE keeps the HAM clock at 2.4 GHz.
            o_pair = [pp.tile([128, Q], BF16, tag=f"oh{i}", name=f"o_pair{i}")
                      for i in range(2)]
            first_panel = True
            for qp in range(Q // 1024):
                q0 = qp * 1024
                for h in range(HL):
                    tl, po = h // 2, (h % 2) * 64
                    oA = psO.tile([65, 512], F32, tag="oA", name="oA")
                    oB = psO.tile([65, 512], F32, tag="oB", name="oB")

                    def attnv(p, kt, h=h, oA=oA, oB=oB):
                        for hf, o_ps in enumerate((oA, oB)):
                            nc.tensor.matmul(
                                o_ps[:],
                                (v_sb[kt][:, h * 65:h * 65 + 65]),
                                (p[:, hf * 512:(hf + 1) * 512]),
                                start=(kt == 0), stop=(kt == KTC - 1))

                    prev = None
                    for kt in range(KTC):
                        if first_panel:
                            vproj(kt)
                        s_ps = psS.tile([128, 1024], F32, tag="s", name="s_ps")
                        for hf in range(2):
                            nc.tensor.matmul(
                                s_ps[:, hf * 512:(hf + 1) * 512],
                                (k_t[tl][po:po + 64, kt * 128:(kt + 1) * 128]),
                                (q_t[tl][po:po + 64,
                                         q0 + hf * 512:q0 + (hf + 1) * 512]),
                                start=True, stop=True)
                        p_sb = cb.tile([128, 1024], BF16, tag="p", bufs=4,
                                       name="p_sb")
                        nc.scalar.activation(
                            p_sb[:], s_ps[:], EXP,
                            bias=mask_sb[:, kt:kt + 1], scale=1.0)
                        if prev is not None:
                            attnv(*prev)
                        prev = (p_sb, kt)
                    attnv(*prev)
                    first_panel = False
                    # normalize: o[dh, q] /= denom[q] (denom is o_ps row 64):
                    # stage denom in SBUF, broadcast over 64 partitions via a
                    # C=1 matmul, 64-lane fast reciprocal, then scale.
                    for hf, o_ps in enumerate((oA, oB)):
                        dn = cb.tile([65, 512], F32R, tag="dn", bufs=2,
                                     name="dn")
                        nc.vector.tensor_copy(dn[64:65, :], o_ps[64:65, :])
                        bc_ps = psA.tile([64, 512], F32, tag="proj",
                                         name="bc_ps")
                        nc.tensor.matmul(bc_ps[:], (ones_sb[64:65, :]),
                                         (dn[64:65, :]), start=True, stop=True)
                        inv_sb = cb.tile([64, 512], F32, tag="invb", bufs=2,
                                         name="inv_sb")
                        nc.vector.reciprocal_approx_fast(inv_sb[:], bc_ps[:])
                        cols = slice(q0 + hf * 512, q0 + (hf + 1) * 512)
                        if h % 2 == 0:
                            nc.vector.tensor_mul(
                                o_pair[tl][0:64, cols], o_ps[0:64, :],
                                inv_sb[:])
                        else:
                            # DVE lanes can't cross partitions; normalize into
                            # a scratch tile and DMA-hop it to partitions
                            # 64-127 so the output projection can run C=128
                            o_tmp = cb.tile([64, 512], BF16, tag="otmp",
                                            bufs=2, name="o_tmp")
                            nc.vector.tensor_mul(o_tmp[:], o_ps[0:64, :],
                                                 inv_sb[:])
                            nc.sync.dma_start(o_pair[tl][64:128, cols],
                                              o_tmp[:])

                # ---- output projection for this q-panel, on the psA slots
                # (idle during attention), overlapping the next panel ----
                for ot in range(4):
                    for qs in (2 * qp, 2 * qp + 1):
                        y_ps = psA.tile([128, 512], F32, tag="proj", name="ps")
                        for pr in range(2):
                            nc.tensor.matmul(
                                y_ps[:],
                                (wo[pr][:, ot * 128:(ot + 1) * 128]),
                                (o_pair[pr][:, qs * 512:(qs + 1) * 512]),
                                start=(pr == 0), stop=(pr == 1))
                        y_sb = cb.tile([128, 512], F32, tag="y", bufs=2,
                                       name="y_sb")
                        nc.vector.tensor_copy(y_sb[:], y_ps[:])
                        nc.sync.dma_start(
                            y_d[ot * 128:(ot + 1) * 128,
                                qs * 512:(qs + 1) * 512],
                            y_sb[:])

    nc.compile()
    return nc


def make_in_maps(queries, keys, values, valid_lens, W_q, W_k, W_v, W_o, KT):
    queries = np.asarray(queries, np.float32)
    keys = np.asarray(keys, np.float32)
    values = np.asarray(values, np.float32)
    W_q = np.asarray(W_q, np.float32)
    W_k = np.asarray(W_k, np.float32)
    W_v = np.asarray(W_v, np.float32)
    W_o = np.asarray(W_o, np.float32)
    vl = np.asarray(valid_lens).astype(np.int64)
    in_maps = []
    for c in range(N_CORES):
        b, hg = c // 2, c % 2
        sl = slice(hg * DL, (hg + 1) * DL)
        m = np.where(np.arange(KT) < vl[b], 0.0, NEG).astype(np.float32)
        bf = ml_dtypes.bfloat16
        in_maps.append({
            "xq_t": np.ascontiguousarray(queries[b].T).astype(bf),
            "xk_t": np.ascontiguousarray(keys[b, :KT].T).astype(bf),
            "xv_t": np.ascontiguousarray(values[b, :KT].T).astype(bf),
            "wq_t": np.ascontiguousarray((W_q[sl, :] / 8.0).T).astype(bf),
            "wk_t": np.ascontiguousarray(W_k[sl, :].T).astype(bf),
            "wv_t": np.ascontiguousarray(W_v[sl, :].T).astype(bf),
            "wo_t": np.ascontiguousarray(W_o[:, sl].T).astype(bf),
            "mask": np.ascontiguousarray(m.reshape(KT // 128, 128).T),
        })
    return in_maps


def pick_kt(valid_lens):
    vl_max = int(np.asarray(valid_lens).max())
    return int(min(KSEQ, max(128, ((vl_max + 127) // 128) * 128)))


def kernel(queries, keys, values, valid_lens, W_q, W_k, W_v, W_o):
    KT = pick_kt(valid_lens)
    nc = build_nc(KT)
    in_maps = make_in_maps(queries, keys, values, valid_lens,
                           W_q, W_k, W_v, W_o, KT)
    res = run_bass_kernel_spmd(nc, in_maps, list(range(N_CORES))).results
    out = np.empty((B, Q, D), np.float32)
    for b in range(B):
        out[b] = (res[2 * b]["y_t"] + res[2 * b + 1]["y_t"]).T
    return out



# revision 3
# speedup vs baseline: 1.3820x; 1.3820x over previous
"""Multi-head attention (B=4, Q=K=2048, D=512, H=8) on 8 TRN2 NeuronCores.

Sharding: every core runs the SAME program but a different (head-pair, q-half)
of every batch: core c owns heads {2*(c%4), 2*(c%4)+1} and query window
[1024*(c//4), 1024*(c//4)+1024) of ALL four batches.  Each batch is truncated
to its OWN KT_b = ceil128(valid_len[b]) -- key positions beyond valid_len have
softmax weight exactly 0, so per-batch truncation is exact and cuts total
attention work from 4*max(KT) to sum(KT).  Every core then processes exactly
sum_b KT_b/128 key-chunks: perfectly balanced by construction.

Device-side choices:
  * Activations transposed ([feature, seq]); matmuls contract the partition dim.
  * Scores computed transposed (S_T[k, q] = K_h @ Q_h^T).  The two heads of a
    core's pair sit on partitions 0-63 / 64-127 of shared q_t/k_t tiles, so
    their C=64 score matmuls land on disjoint PE row-groups (auto
    tile_position (0,0)/(64,0)) and run CONCURRENTLY in the array -- 2x score
    throughput vs. sequential heads.
  * No mask and no exp bias at all: chunks are either fully valid or the final
    partial chunk, whose invalid key rows are zeroed in v_sb (values AND the
    interleaved ones column), removing them from both the attnV numerator and
    the softmax denominator.  One exp then covers both heads' score tiles
    ([128, 1024] PSUM spanning the pair's two banks).
  * Ones-column interleaved into v gives the softmax denominator for free
    (row 64 of each head's [65, 512] attnV accumulator).
  * bf16 matmul pipeline with fp32 PSUM; softmax/normalization fp32.
  * Partial-output projection per (core, batch); host sums the 4 head-pair
    partials per (batch, q-half).  Partials in bf16 to halve output DMA.
  * Projections of unit u+1 and output-projections of unit u-1 are emitted as
    filler tasks inside unit u's attention loop so the PE never idles while
    the ACT engine (the attention-phase bottleneck) chews exps.
"""

import functools
from collections import deque

import ml_dtypes
import numpy as np

import concourse.bacc as bacc
import concourse.bass as bass
import concourse.mybir as mybir
from concourse import tile
from concourse.bass_utils import run_bass_kernel_spmd

F32 = mybir.dt.float32
F32R = mybir.dt.float32r
BF16 = mybir.dt.bfloat16

B, Q, KSEQ, D, H = 4, 2048, 2048, 512, 8
DH = D // H          # 64   head dim
QW = 1024            # per-core query window
N_CORES = 8
EXP = mybir.ActivationFunctionType.Exp


@functools.lru_cache(maxsize=4)
def build_nc(kts, vls):
    """One SPMD program; kts/vls are the per-unit (ascending-KT-ordered)
    key lengths / valid lens of the 4 batches."""
    assert all(kt % 128 == 0 and 128 <= kt <= KSEQ for kt in kts)
    NCH = [kt // 128 for kt in kts]
    KOFF = np.concatenate([[0], np.cumsum(kts)]).tolist()
    SK = KOFF[-1]

    nc = bacc.Bacc("TRN2", target_bir_lowering=False, debug=False,
                   num_devices=N_CORES)

    def din(name, shape, dt=BF16):
        return nc.dram_tensor(name, shape, dt, kind="ExternalInput").ap()

    xq_d = din("xq_t", [D, 4 * QW])
    xk_d = din("xk_t", [D, SK])
    xv_d = din("xv_t", [D, SK])
    wq_d = din("wq_t", [D, 128])
    wk_d = din("wk_t", [D, 128])
    wv_d = din("wv_t", [D, 128])
    wo_d = din("wo_t", [128, D])
    y_d = nc.dram_tensor("y_t", [D, 4 * QW], BF16, kind="ExternalOutput").ap()

    with tile.TileContext(nc) as tc:
        with (
            nc.allow_low_precision(reason="bf16 matmul operands"),
            tc.tile_pool(name="persist", bufs=1) as pp,
            tc.tile_pool(name="cbuf", bufs=1) as cb,
            # 8 PSUM banks: psS 2x[128,1024] score tiles (pair x 512q),
            # psO oA+oB [65,512] attnV accumulators, psA 2x[128,512]
            # projections / denominator broadcast.
            tc.tile_pool(name="psS", bufs=2, space=bass.MemorySpace.PSUM) as psS,
            tc.tile_pool(name="psO", bufs=1, space=bass.MemorySpace.PSUM) as psO,
            tc.tile_pool(name="psA", bufs=2, space=bass.MemorySpace.PSUM) as psA,
        ):
            # ---- persistent tiles ----
            wq = [pp.tile([128, 128], BF16, tag=f"wq{i}", name=f"wq{i}") for i in range(4)]
            wk = [pp.tile([128, 128], BF16, tag=f"wk{i}", name=f"wk{i}") for i in range(4)]
            wv = [pp.tile([128, 128], BF16, tag=f"wv{i}", name=f"wv{i}") for i in range(4)]
            wo = pp.tile([128, D], BF16, tag="wo", name="wo")
            onescr = pp.tile([128, DH], F32, tag="onescr", name="onescr")
            ones_sb = pp.tile([65, DH], F32R, tag="ones", name="ones_sb")
            actwarm = pp.tile([1, 1], F32, tag="actwarm", name="actwarm")

            xq = [[pp.tile([128, QW], BF16, tag=f"xq{u}_{i}", name=f"xq{u}_{i}")
                   for i in range(4)] for u in range(4)]
            xk = [[pp.tile([128, kts[u]], BF16, tag=f"xk{u}_{i}", name=f"xk{u}_{i}")
                   for i in range(4)] for u in range(4)]
            xv = [[pp.tile([128, kts[u]], BF16, tag=f"xv{u}_{i}", name=f"xv{u}_{i}")
                   for i in range(4)] for u in range(4)]
            q_t = [pp.tile([128, QW], BF16, tag=f"q_t{u}", name=f"q_t{u}")
                   for u in range(4)]
            k_t = [pp.tile([128, kts[u]], BF16, tag=f"k_t{u}", name=f"k_t{u}")
                   for u in range(4)]
            v_sb = [[pp.tile([128, 130], BF16, tag=f"v{u}_{kt}", name=f"v{u}_{kt}")
                     for kt in range(NCH[u])] for u in range(4)]

            # ---- DMAs, unit-0-first so its projections start ASAP ----
            for i in range(4):
                nc.sync.dma_start(wq[i][:], wq_d[i * 128:(i + 1) * 128, :])
            # pull the ACT exp table load into the initial DMA wait
            nc.vector.memset(onescr[:], 1.0)
            nc.scalar.activation(actwarm[:], onescr[0:1, 0:1], EXP)
            nc.vector.tensor_copy(ones_sb[64:65, :], onescr[64:65, :])
            for i in range(4):
                nc.sync.dma_start(xq[0][i][:], xq_d[i * 128:(i + 1) * 128, 0:QW])
            for i in range(4):
                nc.sync.dma_start(wk[i][:], wk_d[i * 128:(i + 1) * 128, :])
            for i in range(4):
                nc.sync.dma_start(xk[0][i][:], xk_d[i * 128:(i + 1) * 128,
                                                    KOFF[0]:KOFF[1]])
            for i in range(4):
                nc.sync.dma_start(wv[i][:], wv_d[i * 128:(i + 1) * 128, :])
            for i in range(4):
                nc.sync.dma_start(xv[0][i][:], xv_d[i * 128:(i + 1) * 128,
                                                    KOFF[0]:KOFF[1]])
            nc.sync.dma_start(wo[:], wo_d[:])
            for u in range(1, 4):
                for i in range(4):
                    nc.sync.dma_start(xq[u][i][:],
                                      xq_d[i * 128:(i + 1) * 128,
                                           u * QW:(u + 1) * QW])
                for i in range(4):
                    nc.sync.dma_start(xk[u][i][:],
                                      xk_d[i * 128:(i + 1) * 128,
                                           KOFF[u]:KOFF[u + 1]])
                for i in range(4):
                    nc.sync.dma_start(xv[u][i][:],
                                      xv_d[i * 128:(i + 1) * 128,
                                           KOFF[u]:KOFF[u + 1]])

            # ---- projection / output-projection task factories ----
            def qproj(u, qs):
                def run():
                    ps = psA.tile([128, 512], F32, tag="proj", name="ps")
                    for ic in range(4):
                        nc.tensor.matmul(ps[:], wq[ic][:],
                                         xq[u][ic][:, qs * 512:(qs + 1) * 512],
                                         start=(ic == 0), stop=(ic == 3))
                    nc.vector.tensor_copy(q_t[u][:, qs * 512:(qs + 1) * 512],
                                          ps[:])
                return run

            def kproj(u, s, w):
                def run():
                    ps = psA.tile([128, 512], F32, tag="proj", name="ps")
                    for ic in range(4):
                        nc.tensor.matmul(ps[:, :w], wk[ic][:],
                                         xk[u][ic][:, s:s + w],
                                         start=(ic == 0), stop=(ic == 3))
                    nc.vector.tensor_copy(k_t[u][:, s:s + w], ps[:, :w])
                return run

            def vproj(u, kt):
                nv = min(128, vls[u] - kt * 128)  # valid rows in this chunk

                def run():
                    ps = psA.tile([128, 512], F32, tag="proj", name="ps")
                    for ic in range(4):
                        nc.tensor.matmul(ps[:, 0:128],
                                         xv[u][ic][:, kt * 128:(kt + 1) * 128],
                                         wv[ic][:],
                                         start=(ic == 0), stop=(ic == 3))
                    t = v_sb[u][kt]
                    if nv < 128:
                        nc.vector.memset(t[:], 0.0)
                    nc.vector.tensor_copy(t[0:nv, 64::65], onescr[0:nv, 0:2])
                    for h in range(2):
                        nc.vector.tensor_copy(t[0:nv, h * 65:h * 65 + 64],
                                              ps[0:nv, h * 64:(h + 1) * 64])
                return run

            def oproj(u, ot, qs):
                def run():
                    ps = psA.tile([128, 512], F32, tag="proj", name="ps")
                    nc.tensor.matmul(ps[:], wo[:, ot * 128:(ot + 1) * 128],
                                     o_pair[u][:, qs * 512:(qs + 1) * 512],
                                     start=True, stop=True)
                    y_sb = cb.tile([128, 512], BF16, tag="y", bufs=2,
                                   name="y_sb")
                    nc.vector.tensor_copy(y_sb[:], ps[:])
                    nc.sync.dma_start(
                        y_d[ot * 128:(ot + 1) * 128,
                            u * QW + qs * 512:u * QW + (qs + 1) * 512],
                        y_sb[:])
                return run

            def proj_tasks(u):
                t = [("proj", u, qproj(u, 0)), ("proj", u, qproj(u, 1))]
                for s in range(0, kts[u], 512):
                    t.append(("proj", u, kproj(u, s, min(512, kts[u] - s))))
                for kt in range(NCH[u]):
                    t.append(("proj", u, vproj(u, kt)))
                return t

            o_pair = {}

            # ---- flat attention pipeline over (unit, ip, kt) steps ----
            fillers = deque()
            fillers.extend(proj_tasks(1))

            for f in proj_tasks(0):   # unit 0 projections inline
                f[2]()

            steps = [(u, ip, kt) for u in range(4) for ip in range(2)
                     for kt in range(NCH[u])]

            def scores(u, ip, kt):
                s_ps = psS.tile([128, 1024], F32, tag="s", name="s_ps")
                for hf in range(2):
                    nc.tensor.matmul(
                        s_ps[:, hf * 512:(hf + 1) * 512],
                        k_t[u][hf * 64:(hf + 1) * 64, kt * 128:(kt + 1) * 128],
                        q_t[u][hf * 64:(hf + 1) * 64,
                               ip * 512:(ip + 1) * 512],
                        start=True, stop=True)
                p_sb = cb.tile([128, 1024], BF16, tag="p", bufs=4, name="p_sb")
                nc.scalar.activation(p_sb[:], s_ps[:], EXP, scale=1.0)
                return p_sb

            acc = {}

            def attnv(u, ip, kt, p_sb):
                if kt == 0:
                    acc["oA"] = psO.tile([65, 512], F32, tag="oA", name="oA")
                    acc["oB"] = psO.tile([65, 512], F32, tag="oB", name="oB")
                for hf, o_ps in enumerate((acc["oA"], acc["oB"])):
                    nc.tensor.matmul(
                        o_ps[:], v_sb[u][kt][:, hf * 65:hf * 65 + 65],
                        p_sb[:, hf * 512:(hf + 1) * 512],
                        start=(kt == 0), stop=(kt == NCH[u] - 1))

            def normalize(u, ip):
                if u not in o_pair:
                    o_pair[u] = cb.tile([128, QW], BF16, tag="o", bufs=2,
                                        name="o_pair")
                cols = slice(ip * 512, (ip + 1) * 512)
                for hf, o_ps in enumerate((acc["oA"], acc["oB"])):
                    dn = cb.tile([65, 512], F32R, tag="dn", bufs=2, name="dn")
                    nc.vector.tensor_copy(dn[64:65, :], o_ps[64:65, :])
                    bc_ps = psA.tile([64, 512], F32, tag="proj", name="bc_ps")
                    nc.tensor.matmul(bc_ps[:], ones_sb[64:65, :],
                                     dn[64:65, :], start=True, stop=True)
                    inv_sb = cb.tile([64, 512], F32, tag="invb", bufs=2,
                                     name="inv_sb")
                    nc.vector.reciprocal_approx_fast(inv_sb[:], bc_ps[:])
                    if hf == 0:
                        nc.vector.tensor_mul(o_pair[u][0:64, cols],
                                             o_ps[0:64, :], inv_sb[:])
                    else:
                        # DVE lanes can't cross partitions: normalize into a
                        # scratch tile, DMA-hop to partitions 64-127
                        o_tmp = cb.tile([64, 512], BF16, tag="otmp", bufs=2,
                                        name="o_tmp")
                        nc.vector.tensor_mul(o_tmp[:], o_ps[0:64, :],
                                             inv_sb[:])
                        nc.sync.dma_start(o_pair[u][64:128, cols], o_tmp[:])

            prev = None
            for step in steps:
                u, ip, kt = step
                if ip == 0 and kt == 0 and u > 0:
                    # everything feeding unit u must precede its scores on
                    # the PE stream (in-order queue => would deadlock after)
                    while any(t[0] == "proj" and t[1] <= u for t in fillers):
                        fillers.popleft()[2]()
                p = scores(u, ip, kt)
                if prev is not None:
                    pu, pip, pkt = prev[0]
                    attnv(pu, pip, pkt, prev[1])
                    if pkt == NCH[pu] - 1:
                        normalize(pu, pip)
                        if pip == 1:
                            for ot in range(4):
                                for qs in range(2):
                                    fillers.append(("oproj", pu,
                                                    oproj(pu, ot, qs)))
                            if pu + 2 <= 3:
                                fillers.extend(proj_tasks(pu + 2))
                if fillers:
                    fillers.popleft()[2]()
                prev = (step, p)
            pu, pip, pkt = prev[0]
            attnv(pu, pip, pkt, prev[1])
            normalize(pu, pip)
            while fillers:
                fillers.popleft()[2]()
            for ot in range(4):
                for qs in range(2):
                    oproj(3, ot, qs)()

    nc.compile()
    return nc


def ceil128(v):
    return int(min(KSEQ, max(128, ((int(v) + 127) // 128) * 128)))


def plan(valid_lens):
    vl = np.asarray(valid_lens).astype(np.int64)
    order = sorted(range(B), key=lambda b: (ceil128(vl[b]), b))
    kts = tuple(ceil128(vl[b]) for b in order)
    vls = tuple(int(min(vl[b], kts[i])) for i, b in enumerate(order))
    return order, kts, vls


def make_in_maps(queries, keys, values, valid_lens, W_q, W_k, W_v, W_o,
                 order, kts):
    bf = ml_dtypes.bfloat16
    queries = np.asarray(queries, np.float32)
    keys = np.asarray(keys, np.float32)
    values = np.asarray(values, np.float32)
    W_q = np.asarray(W_q, np.float32)
    W_k = np.asarray(W_k, np.float32)
    W_v = np.asarray(W_v, np.float32)
    W_o = np.asarray(W_o, np.float32)
    in_maps = []
    for c in range(N_CORES):
        hp, qp = c % 4, c // 4
        fsl = slice(hp * 128, (hp + 1) * 128)
        qsl = slice(qp * QW, (qp + 1) * QW)
        xq = np.concatenate([queries[b, qsl, :].T for b in order], axis=1)
        xk = np.concatenate([keys[b, :kts[i], :].T
                             for i, b in enumerate(order)], axis=1)
        xv = np.concatenate([values[b, :kts[i], :].T
                             for i, b in enumerate(order)], axis=1)
        in_maps.append({
            "xq_t": np.ascontiguousarray(xq).astype(bf),
            "xk_t": np.ascontiguousarray(xk).astype(bf),
            "xv_t": np.ascontiguousarray(xv).astype(bf),
            "wq_t": np.ascontiguousarray((W_q[fsl, :] / 8.0).T).astype(bf),
            "wk_t": np.ascontiguousarray(W_k[fsl, :].T).astype(bf),
            "wv_t": np.ascontiguousarray(W_v[fsl, :].T).astype(bf),
            "wo_t": np.ascontiguousarray(W_o[:, fsl].T).astype(bf),
        })
    return in_maps


def assemble(results, order):
    out = np.empty((B, Q, D), np.float32)
    for qp in range(2):
        for i, b in enumerate(order):
            acc = np.zeros((D, QW), np.float32)
            for hp in range(4):
                acc += np.asarray(results[qp * 4 + hp]["y_t"]
                                  [:, i * QW:(i + 1) * QW], np.float32)
            out[b, qp * QW:(qp + 1) * QW, :] = acc.T
    return out


def kernel(queries, keys, values, valid_lens, W_q, W_k, W_v, W_o):
    order, kts, vls = plan(valid_lens)
    nc = build_nc(kts, vls)
    in_maps = make_in_maps(queries, keys, values, valid_lens,
                           W_q, W_k, W_v, W_o, order, kts)
    res = run_bass_kernel_spmd(nc, in_maps, list(range(N_CORES))).results
    return assemble(res, order)


# revision 4
# speedup vs baseline: 1.5057x; 1.0895x over previous
"""Multi-head attention (B=4, Q=K=2048, D=512, H=8) on 8 TRN2 NeuronCores.

Sharding: every core runs the SAME program but a different (head-pair, q-half)
of every batch: core c owns heads {2*(c%4), 2*(c%4)+1} and query window
[1024*(c//4), 1024*(c//4)+1024) of ALL four batches.  Each batch is truncated
to its OWN KT_b = ceil128(valid_len[b]) -- key positions beyond valid_len have
softmax weight exactly 0, so per-batch truncation is exact and cuts total
attention work from 4*max(KT) to sum(KT).  Every core then processes exactly
sum_b KT_b/128 key-chunks: perfectly balanced by construction.

Device-side choices:
  * Activations transposed ([feature, seq]); matmuls contract the partition dim.
  * Scores computed transposed (S_T[k, q] = K_h @ Q_h^T).  The two heads of a
    core's pair sit on partitions 0-63 / 64-127 of shared q_t/k_t tiles, so
    their C=64 score matmuls land on disjoint PE row-groups (auto
    tile_position (0,0)/(64,0)) and run CONCURRENTLY in the array -- 2x score
    throughput vs. sequential heads.
  * No mask and no exp bias: chunks are either fully valid or the final
    partial chunk, whose invalid key rows are zeroed in v_sb (values AND the
    interleaved ones column), removing them from both the attnV numerator and
    the softmax denominator.  One exp covers both heads' score tiles
    ([128, 1024] PSUM spanning the pair's two banks).
  * Ones-column interleaved into v gives the softmax denominator for free
    (row 64 of each head's [65, 512] attnV accumulator).
  * All inputs arrive pre-packed host-side as [128, N] panels so each tensor
    is ONE large DMA descriptor (16 input DMAs total) -- the sync engine's
    per-descriptor issue cost otherwise starves the front of the kernel.
  * v_sb is a 3D [128, NCH, 130] tile per unit: V-proj runs 4 key-chunks per
    PSUM tile and lands them with two strided 3D casts instead of 8 small
    copies; ones columns are memset once per unit in the prologue.
  * bf16 matmul pipeline with fp32 PSUM; softmax/normalization fp32.
  * Partial-output projection per (core, batch); host sums the 4 head-pair
    partials per (batch, q-half).  Partials in bf16 to halve output DMA.
  * Projections of later units and output-projections of earlier units are
    emitted as filler tasks inside the attention loop so the PE never idles
    while the ACT engine (the attention-phase bottleneck) chews exps.
  * Units run in ascending-KT order rotated so the smallest unit comes last:
    the first unit's DMA is small (fast start) and the last unit's
    output-projection tail is short.
"""

import functools
from collections import deque

import ml_dtypes
import numpy as np

import concourse.bacc as bacc
import concourse.bass as bass
import concourse.mybir as mybir
from concourse import tile
from concourse.bass_utils import run_bass_kernel_spmd

F32 = mybir.dt.float32
F32R = mybir.dt.float32r
BF16 = mybir.dt.bfloat16

B, Q, KSEQ, D, H = 4, 2048, 2048, 512, 8
DH = D // H          # 64   head dim
QW = 1024            # per-core query window
N_CORES = 8
EXP = mybir.ActivationFunctionType.Exp


@functools.lru_cache(maxsize=4)
def build_nc(kts, vls):
    """One SPMD program; kts/vls are the per-unit (execution-ordered)
    key lengths / valid lens of the 4 batches."""
    assert all(kt % 128 == 0 and 128 <= kt <= KSEQ for kt in kts)
    NCH = [kt // 128 for kt in kts]
    KOFF = np.concatenate([[0], np.cumsum([4 * k for k in kts])]).tolist()
    SK4 = KOFF[-1]

    nc = bacc.Bacc("TRN2", target_bir_lowering=False, debug=False,
                   num_devices=N_CORES)

    def din(name, shape, dt=BF16):
        return nc.dram_tensor(name, shape, dt, kind="ExternalInput").ap()

    xq_d = din("xq_t", [128, 16 * QW])
    xk_d = din("xk_t", [128, SK4])
    xv_d = din("xv_t", [128, SK4])
    wq_d = din("wq_t", [128, D])
    wk_d = din("wk_t", [128, D])
    wv_d = din("wv_t", [128, D])
    wo_d = din("wo_t", [128, D])
    y_d = nc.dram_tensor("y_t", [D, 4 * QW], BF16, kind="ExternalOutput").ap()

    with tile.TileContext(nc) as tc:
        with (
            nc.allow_low_precision(reason="bf16 matmul operands"),
            tc.tile_pool(name="persist", bufs=1) as pp,
            tc.tile_pool(name="cbuf", bufs=1) as cb,
            # 8 PSUM banks: psS 2x[128,1024] score tiles (pair x 512q),
            # psO oA+oB [65,512] attnV accumulators, psA 2x[128,512]
            # projections / denominator broadcast.
            tc.tile_pool(name="psS", bufs=2, space=bass.MemorySpace.PSUM) as psS,
            tc.tile_pool(name="psO", bufs=1, space=bass.MemorySpace.PSUM) as psO,
            tc.tile_pool(name="psA", bufs=2, space=bass.MemorySpace.PSUM) as psA,
        ):
            # ---- persistent tiles ----
            wq = pp.tile([128, D], BF16, tag="wq", name="wq")
            wk = pp.tile([128, D], BF16, tag="wk", name="wk")
            wv = pp.tile([128, D], BF16, tag="wv", name="wv")
            wo = pp.tile([128, D], BF16, tag="wo", name="wo")
            onescr = pp.tile([128, DH], F32, tag="onescr", name="onescr")
            ones_sb = pp.tile([65, DH], F32R, tag="ones", name="ones_sb")
            actwarm = pp.tile([1, 1], F32, tag="actwarm", name="actwarm")

            xq = [pp.tile([128, 4 * QW], BF16, tag=f"xq{u}", name=f"xq{u}")
                  for u in range(4)]
            xk = [pp.tile([128, 4 * kts[u]], BF16, tag=f"xk{u}", name=f"xk{u}")
                  for u in range(4)]
            xv = [pp.tile([128, 4 * kts[u]], BF16, tag=f"xv{u}", name=f"xv{u}")
                  for u in range(4)]
            q_t = [pp.tile([128, QW], BF16, tag=f"q_t{u}", name=f"q_t{u}")
                   for u in range(4)]
            k_t = [pp.tile([128, kts[u]], BF16, tag=f"k_t{u}", name=f"k_t{u}")
                   for u in range(4)]
            v_sb = [pp.tile([128, NCH[u], 130], BF16, tag=f"v{u}",
                            name=f"v{u}") for u in range(4)]

            # ---- DMAs, first-unit-first so its projections start ASAP ----
            nc.sync.dma_start(wq[:], wq_d[:])
            # pull the ACT exp table load into the initial DMA wait
            nc.vector.memset(onescr[:], 1.0)
            nc.scalar.activation(actwarm[:], onescr[0:1, 0:1], EXP)
            nc.vector.tensor_copy(ones_sb[64:65, :], onescr[64:65, :])
            # prologue memsets: ones columns for every unit's v_sb; zero the
            # final partial chunk first so its invalid rows stay zero.
            for u in range(4):
                nv = vls[u] - (NCH[u] - 1) * 128
                if nv < 128:
                    if NCH[u] > 1:
                        nc.vector.memset(v_sb[u][:, 0:NCH[u] - 1, 64::65], 1.0)
                    nc.vector.memset(v_sb[u][:, NCH[u] - 1, :], 0.0)
                    nc.vector.memset(v_sb[u][0:nv, NCH[u] - 1, 64::65], 1.0)
                else:
                    nc.vector.memset(v_sb[u][:, :, 64::65], 1.0)
            nc.sync.dma_start(wk[:], wk_d[:])
            nc.sync.dma_start(xk[0][:], xk_d[:, KOFF[0]:KOFF[1]])
            nc.sync.dma_start(wv[:], wv_d[:])
            nc.sync.dma_start(xv[0][:], xv_d[:, KOFF[0]:KOFF[1]])
            nc.sync.dma_start(xq[0][:], xq_d[:, 0:4 * QW])
            nc.sync.dma_start(wo[:], wo_d[:])
            for u in range(1, 4):
                nc.sync.dma_start(xk[u][:], xk_d[:, KOFF[u]:KOFF[u + 1]])
                nc.sync.dma_start(xv[u][:], xv_d[:, KOFF[u]:KOFF[u + 1]])
                nc.sync.dma_start(xq[u][:],
                                  xq_d[:, u * 4 * QW:(u + 1) * 4 * QW])

            # ---- projection / output-projection task factories ----
            def qproj(u, qs):
                def run():
                    ps = psA.tile([128, 512], F32, tag="proj", name="ps")
                    for ic in range(4):
                        nc.tensor.matmul(
                            ps[:], wq[:, ic * 128:(ic + 1) * 128],
                            xq[u][:, ic * QW + qs * 512:
                                  ic * QW + (qs + 1) * 512],
                            start=(ic == 0), stop=(ic == 3))
                    nc.vector.tensor_copy(q_t[u][:, qs * 512:(qs + 1) * 512],
                                          ps[:])
                return run

            def kproj(u, s, w):
                kt = kts[u]

                def run():
                    ps = psA.tile([128, 512], F32, tag="proj", name="ps")
                    for ic in range(4):
                        nc.tensor.matmul(ps[:, :w],
                                         wk[:, ic * 128:(ic + 1) * 128],
                                         xk[u][:, ic * kt + s:ic * kt + s + w],
                                         start=(ic == 0), stop=(ic == 3))
                    nc.vector.tensor_copy(k_t[u][:, s:s + w], ps[:, :w])
                return run

            def vproj(u, g):
                # one group = up to 4 key-chunks through a [128, 4, 128] PSUM
                # tile, landed with two strided 3D casts per head
                kt = kts[u]
                kcs = list(range(g * 4, min(NCH[u], g * 4 + 4)))
                nfull = sum(1 for kc in kcs if vls[u] - kc * 128 >= 128)

                def run():
                    ps = psA.tile([128, 4, 128], F32, tag="proj", name="ps")
                    for j, kc in enumerate(kcs):
                        for ic in range(4):
                            nc.tensor.matmul(
                                ps[:, j, :],
                                xv[u][:, ic * kt + kc * 128:
                                      ic * kt + (kc + 1) * 128],
                                wv[:, ic * 128:(ic + 1) * 128],
                                start=(ic == 0), stop=(ic == 3))
                    for h in range(2):
                        if nfull:
                            nc.vector.tensor_copy(
                                v_sb[u][:, kcs[0]:kcs[0] + nfull,
                                        h * 65:h * 65 + 64],
                                ps[:, 0:nfull, h * 64:(h + 1) * 64])
                    for j, kc in enumerate(kcs[nfull:], start=nfull):
                        nv = vls[u] - kc * 128
                        for h in range(2):
                            nc.vector.tensor_copy(
                                v_sb[u][0:nv, kc, h * 65:h * 65 + 64],
                                ps[0:nv, j, h * 64:(h + 1) * 64])
                return run

            def oproj(u, ot, qs):
                def run():
                    ps = psA.tile([128, 512], F32, tag="proj", name="ps")
                    nc.tensor.matmul(ps[:], wo[:, ot * 128:(ot + 1) * 128],
                                     o_pair[u][:, qs * 512:(qs + 1) * 512],
                                     start=True, stop=True)
                    y_sb = cb.tile([128, 512], BF16, tag="y", bufs=2,
                                   name="y_sb")
                    nc.vector.tensor_copy(y_sb[:], ps[:])
                    nc.sync.dma_start(
                        y_d[ot * 128:(ot + 1) * 128,
                            u * QW + qs * 512:u * QW + (qs + 1) * 512],
                        y_sb[:])
                return run

            def proj_tasks(u):
                t = [("proj", u, qproj(u, 0)), ("proj", u, qproj(u, 1))]
                for s in range(0, kts[u], 512):
                    t.append(("proj", u, kproj(u, s, min(512, kts[u] - s))))
                for g in range((NCH[u] + 3) // 4):
                    t.append(("proj", u, vproj(u, g)))
                return t

            o_pair = {}

            # ---- flat attention pipeline over (unit, ip, kt) steps ----
            fillers = deque()
            fillers.extend(proj_tasks(1))

            for f in proj_tasks(0):   # unit 0 projections inline
                f[2]()

            steps = [(u, ip, kt) for u in range(4) for ip in range(2)
                     for kt in range(NCH[u])]

            def scores(u, ip, kt):
                s_ps = psS.tile([128, 1024], F32, tag="s", name="s_ps")
                for hf in range(2):
                    nc.tensor.matmul(
                        s_ps[:, hf * 512:(hf + 1) * 512],
                        k_t[u][hf * 64:(hf + 1) * 64, kt * 128:(kt + 1) * 128],
                        q_t[u][hf * 64:(hf + 1) * 64,
                               ip * 512:(ip + 1) * 512],
                        start=True, stop=True)
                p_sb = cb.tile([128, 1024], BF16, tag="p", bufs=4, name="p_sb")
                nc.scalar.activation(p_sb[:], s_ps[:], EXP, scale=1.0)
                return p_sb

            acc = {}

            def attnv(u, ip, kt, p_sb):
                if kt == 0:
                    acc["oA"] = psO.tile([65, 512], F32, tag="oA", name="oA")
                    acc["oB"] = psO.tile([65, 512], F32, tag="oB", name="oB")
                for hf, o_ps in enumerate((acc["oA"], acc["oB"])):
                    nc.tensor.matmul(
                        o_ps[:], v_sb[u][:, kt, hf * 65:hf * 65 + 65],
                        p_sb[:, hf * 512:(hf + 1) * 512],
                        start=(kt == 0), stop=(kt == NCH[u] - 1))

            def normalize(u, ip):
                if u not in o_pair:
                    o_pair[u] = cb.tile([128, QW], BF16, tag="o", bufs=2,
                                        name="o_pair")
                cols = slice(ip * 512, (ip + 1) * 512)
                for hf, o_ps in enumerate((acc["oA"], acc["oB"])):
                    dn = cb.tile([65, 512], F32R, tag="dn", bufs=2, name="dn")
                    nc.vector.tensor_copy(dn[64:65, :], o_ps[64:65, :])
                    bc_ps = psA.tile([64, 512], F32, tag="proj", name="bc_ps")
                    nc.tensor.matmul(bc_ps[:], ones_sb[64:65, :],
                                     dn[64:65, :], start=True, stop=True)
                    inv_sb = cb.tile([64, 512], F32, tag="invb", bufs=2,
                                     name="inv_sb")
                    nc.vector.reciprocal_approx_fast(inv_sb[:], bc_ps[:])
                    if hf == 0:
                        nc.vector.tensor_mul(o_pair[u][0:64, cols],
                                             o_ps[0:64, :], inv_sb[:])
                    else:
                        # DVE lanes can't cross partitions: normalize into a
                        # scratch tile, DMA-hop to partitions 64-127
                        o_tmp = cb.tile([64, 512], BF16, tag="otmp", bufs=2,
                                        name="o_tmp")
                        nc.vector.tensor_mul(o_tmp[:], o_ps[0:64, :],
                                             inv_sb[:])
                        nc.sync.dma_start(o_pair[u][64:128, cols], o_tmp[:])

            prev = None
            for step in steps:
                u, ip, kt = step
                if ip == 0 and kt == 0 and u > 0:
                    # everything feeding unit u must precede its scores on
                    # the PE stream (in-order queue => would deadlock after)
                    while any(t[0] == "proj" and t[1] <= u for t in fillers):
                        fillers.popleft()[2]()
                p = scores(u, ip, kt)
                if prev is not None:
                    pu, pip, pkt = prev[0]
                    attnv(pu, pip, pkt, prev[1])
                    if pkt == NCH[pu] - 1:
                        normalize(pu, pip)
                        if pip == 1:
                            for ot in range(4):
                                for qs in range(2):
                                    fillers.append(("oproj", pu,
                                                    oproj(pu, ot, qs)))
                            if pu + 2 <= 3:
                                fillers.extend(proj_tasks(pu + 2))
                if fillers:
                    fillers.popleft()[2]()
                if fillers and fillers[0][0] == "oproj":
                    fillers.popleft()[2]()
                prev = (step, p)
            pu, pip, pkt = prev[0]
            attnv(pu, pip, pkt, prev[1])
            normalize(pu, pip)
            while fillers:
                fillers.popleft()[2]()
            for ot in range(4):
                for qs in range(2):
                    oproj(3, ot, qs)()

    nc.compile()
    return nc


def ceil128(v):
    return int(min(KSEQ, max(128, ((int(v) + 127) // 128) * 128)))


def plan(valid_lens):
    vl = np.asarray(valid_lens).astype(np.int64)
    asc = sorted(range(B), key=lambda b: (ceil128(vl[b]), b))
    # smallest unit last (short output tail), rest ascending (fast start)
    order = [asc[1], asc[2], asc[3], asc[0]]
    kts = tuple(ceil128(vl[b]) for b in order)
    vls = tuple(int(min(vl[b], kts[i])) for i, b in enumerate(order))
    return order, kts, vls


def _pack4(x):
    """[512, N] -> [128, 4N] with the four 128-row blocks side by side."""
    n = x.shape[1]
    return x.reshape(4, 128, n).transpose(1, 0, 2).reshape(128, 4 * n)


def make_in_maps(queries, keys, values, valid_lens, W_q, W_k, W_v, W_o,
                 order, kts):
    bf = ml_dtypes.bfloat16
    queries = np.asarray(queries, np.float32)
    keys = np.asarray(keys, np.float32)
    values = np.asarray(values, np.float32)
    W_q = np.asarray(W_q, np.float32)
    W_k = np.asarray(W_k, np.float32)
    W_v = np.asarray(W_v, np.float32)
    W_o = np.asarray(W_o, np.float32)
    in_maps = []
    for c in range(N_CORES):
        hp, qp = c % 4, c // 4
        fsl = slice(hp * 128, (hp + 1) * 128)
        qsl = slice(qp * QW, (qp + 1) * QW)
        xq = np.concatenate([_pack4(queries[b, qsl, :].T) for b in order],
                            axis=1)
        xk = np.concatenate([_pack4(keys[b, :kts[i], :].T)
                             for i, b in enumerate(order)], axis=1)
        xv = np.concatenate([_pack4(values[b, :kts[i], :].T)
                             for i, b in enumerate(order)], axis=1)
        in_maps.append({
            "xq_t": np.ascontiguousarray(xq).astype(bf),
            "xk_t": np.ascontiguousarray(xk).astype(bf),
            "xv_t": np.ascontiguousarray(xv).astype(bf),
            "wq_t": np.ascontiguousarray(_pack4((W_q[fsl, :] / 8.0).T)).astype(bf),
            "wk_t": np.ascontiguousarray(_pack4(W_k[fsl, :].T)).astype(bf),
            "wv_t": np.ascontiguousarray(_pack4(W_v[fsl, :].T)).astype(bf),
            "wo_t": np.ascontiguousarray(W_o[:, fsl].T).astype(bf),
        })
    return in_maps


def assemble(results, order):
    out = np.empty((B, Q, D), np.float32)
    for qp in range(2):
        for i, b in enumerate(order):
            acc = np.zeros((D, QW), np.float32)
            for hp in range(4):
                acc += np.asarray(results[qp * 4 + hp]["y_t"]
                                  [:, i * QW:(i + 1) * QW], np.float32)
            out[b, qp * QW:(qp + 1) * QW, :] = acc.T
    return out


def kernel(queries, keys, values, valid_lens, W_q, W_k, W_v, W_o):
    order, kts, vls = plan(valid_lens)
    nc = build_nc(kts, vls)
    in_maps = make_in_maps(queries, keys, values, valid_lens,
                           W_q, W_k, W_v, W_o, order, kts)
    res = run_bass_kernel_spmd(nc, in_maps, list(range(N_CORES))).results
    return assemble(res, order)


# revision 8
# speedup vs baseline: 1.5130x; 1.0049x over previous
"""Multi-head attention (B=4, Q=K=2048, D=512, H=8) on 8 TRN2 NeuronCores.

Sharding: every core runs the SAME program but a different (head-pair, q-half)
of every batch: core c owns heads {2*(c%4), 2*(c%4)+1} and query window
[1024*(c//4), 1024*(c//4)+1024) of ALL four batches.  Each batch is truncated
to its OWN KT_b = ceil128(valid_len[b]) -- key positions beyond valid_len have
softmax weight exactly 0, so per-batch truncation is exact and cuts total
attention work from 4*max(KT) to sum(KT).  Every core then processes exactly
sum_b KT_b/128 key-chunks: perfectly balanced by construction.

Device-side choices:
  * Activations transposed ([feature, seq]); matmuls contract the partition dim.
  * Scores computed transposed (S_T[k, q] = K_h @ Q_h^T).  The two heads of a
    core's pair sit on partitions 0-63 / 64-127 of shared q_t/k_t tiles, so
    their C=64 score matmuls land on disjoint PE row-groups (auto
    tile_position (0,0)/(64,0)) and run CONCURRENTLY in the array -- 2x score
    throughput vs. sequential heads.
  * No mask and no exp bias: chunks are either fully valid or the final
    partial chunk, whose invalid key rows are zeroed in v_sb (values AND the
    interleaved ones column), removing them from both the attnV numerator and
    the softmax denominator.  One exp covers both heads' score tiles
    ([128, 1024] PSUM spanning the pair's two banks).
  * Ones-column interleaved into v gives the softmax denominator for free
    (row 64 of each head's [65, 512] attnV accumulator).
  * All inputs arrive pre-packed host-side as [128, N] panels so each tensor
    is ONE large DMA descriptor (16 input DMAs total) -- the sync engine's
    per-descriptor issue cost otherwise starves the front of the kernel.
  * v_sb is a 3D [128, NCH, 130] tile per unit: V-proj runs 4 key-chunks per
    PSUM tile and lands them with two strided 3D casts instead of 8 small
    copies; ones columns are memset once per unit in the prologue.
  * bf16 matmul pipeline with fp32 PSUM; softmax/normalization fp32.
  * Partial-output projection per (core, batch); host sums the 4 head-pair
    partials per (batch, q-half).  Partials in bf16 to halve output DMA.
  * Projections of later units and output-projections of earlier units are
    emitted as filler tasks inside the attention loop so the PE never idles
    while the ACT engine (the attention-phase bottleneck) chews exps.
  * Units run in ascending-KT order rotated so the smallest unit comes last:
    the first unit's DMA is small (fast start) and the last unit's
    output-projection tail is short.
"""

import functools
from collections import deque

import ml_dtypes
import numpy as np

import concourse.bacc as bacc
import concourse.bass as bass
import concourse.mybir as mybir
from concourse import tile
from concourse.bass_utils import run_bass_kernel_spmd

F32 = mybir.dt.float32
F32R = mybir.dt.float32r
BF16 = mybir.dt.bfloat16

B, Q, KSEQ, D, H = 4, 2048, 2048, 512, 8
DH = D // H          # 64   head dim
QW = 1024            # per-core query window
N_CORES = 8
EXP = mybir.ActivationFunctionType.Exp


@functools.lru_cache(maxsize=4)
def build_nc(kts, vls):
    """One SPMD program; kts/vls are the per-unit (execution-ordered)
    key lengths / valid lens of the 4 batches."""
    assert all(kt % 128 == 0 and 128 <= kt <= KSEQ for kt in kts)
    NCH = [kt // 128 for kt in kts]
    KOFF = np.concatenate([[0], np.cumsum([4 * k for k in kts])]).tolist()
    SK4 = KOFF[-1]

    nc = bacc.Bacc("TRN2", target_bir_lowering=False, debug=False,
                   num_devices=N_CORES)

    def din(name, shape, dt=BF16):
        return nc.dram_tensor(name, shape, dt, kind="ExternalInput").ap()

    xq_d = din("xq_t", [128, 16 * QW])
    xk_d = din("xk_t", [128, SK4])
    xv_d = din("xv_t", [128, SK4])
    wq_d = din("wq_t", [128, D])
    wk_d = din("wk_t", [128, D])
    wv_d = din("wv_t", [128, D])
    wo_d = din("wo_t", [128, D])
    y_d = nc.dram_tensor("y_t", [D, 4 * QW], BF16, kind="ExternalOutput").ap()

    with tile.TileContext(nc) as tc:
        with (
            nc.allow_low_precision(reason="bf16 matmul operands"),
            tc.tile_pool(name="persist", bufs=1) as pp,
            tc.tile_pool(name="cbuf", bufs=1) as cb,
            # 8 PSUM banks: psS 2x[128,1024] score tiles (pair x 512q),
            # psO oA+oB [65,512] attnV accumulators, psA 2x[128,512]
            # projections / denominator broadcast.
            tc.tile_pool(name="psS", bufs=2, space=bass.MemorySpace.PSUM) as psS,
            tc.tile_pool(name="psO", bufs=1, space=bass.MemorySpace.PSUM) as psO,
            tc.tile_pool(name="psA", bufs=2, space=bass.MemorySpace.PSUM) as psA,
        ):
            # ---- persistent tiles ----
            wq = pp.tile([128, D], BF16, tag="wq", name="wq")
            wk = pp.tile([128, D], BF16, tag="wk", name="wk")
            wv = pp.tile([128, D], BF16, tag="wv", name="wv")
            wo = pp.tile([128, D], BF16, tag="wo", name="wo")
            onescr = pp.tile([128, DH], F32, tag="onescr", name="onescr")
            ones_sb = pp.tile([65, DH], F32R, tag="ones", name="ones_sb")
            actwarm = pp.tile([1, 1], F32, tag="actwarm", name="actwarm")

            xq = [pp.tile([128, 4 * QW], BF16, tag=f"xq{u}", name=f"xq{u}")
                  for u in range(4)]
            xk = [pp.tile([128, 4 * kts[u]], BF16, tag=f"xk{u}", name=f"xk{u}")
                  for u in range(4)]
            xv = [pp.tile([128, 4 * kts[u]], BF16, tag=f"xv{u}", name=f"xv{u}")
                  for u in range(4)]
            q_t = [pp.tile([128, QW], BF16, tag=f"q_t{u}", name=f"q_t{u}")
                   for u in range(4)]
            k_t = [pp.tile([128, kts[u]], BF16, tag=f"k_t{u}", name=f"k_t{u}")
                   for u in range(4)]
            v_sb = [pp.tile([128, NCH[u], 130], BF16, tag=f"v{u}",
                            name=f"v{u}") for u in range(4)]

            # ---- DMAs on TWO hardware queues (sync + scalar HWDGE) so the
            # k/v stream and the weights/q stream transfer in parallel ----
            nc.sync.dma_start(wk[:], wk_d[:])
            nc.scalar.dma_start(wq[:], wq_d[:])
            # pull the ACT exp table load into the initial DMA wait
            nc.vector.memset(onescr[:], 1.0)
            nc.scalar.activation(actwarm[:], onescr[0:1, 0:1], EXP)
            nc.vector.tensor_copy(ones_sb[64:65, :], onescr[64:65, :])
            # prologue memsets: ones columns for every unit's v_sb; zero the
            # final partial chunk first so its invalid rows stay zero.
            for u in range(4):
                nv = vls[u] - (NCH[u] - 1) * 128
                if nv < 128:
                    if NCH[u] > 1:
                        nc.vector.memset(v_sb[u][:, 0:NCH[u] - 1, 64::65], 1.0)
                    nc.vector.memset(v_sb[u][:, NCH[u] - 1, :], 0.0)
                    nc.vector.memset(v_sb[u][0:nv, NCH[u] - 1, 64::65], 1.0)
                else:
                    nc.vector.memset(v_sb[u][:, :, 64::65], 1.0)
            nc.sync.dma_start(xk[0][:], xk_d[:, KOFF[0]:KOFF[1]])
            nc.scalar.dma_start(wv[:], wv_d[:])
            nc.sync.dma_start(xv[0][:], xv_d[:, KOFF[0]:KOFF[1]])
            nc.scalar.dma_start(xq[0][:], xq_d[:, 0:4 * QW])
            nc.scalar.dma_start(wo[:], wo_d[:])
            for u in range(1, 4):
                nc.sync.dma_start(xk[u][:], xk_d[:, KOFF[u]:KOFF[u + 1]])
                nc.sync.dma_start(xv[u][:], xv_d[:, KOFF[u]:KOFF[u + 1]])
                nc.scalar.dma_start(xq[u][:],
                                    xq_d[:, u * 4 * QW:(u + 1) * 4 * QW])

            # ---- projection / output-projection task factories ----
            def qproj(u, qs):
                def run():
                    ps = psA.tile([128, 512], F32, tag="proj", name="ps")
                    for ic in range(4):
                        nc.tensor.matmul(
                            ps[:], wq[:, ic * 128:(ic + 1) * 128],
                            xq[u][:, ic * QW + qs * 512:
                                  ic * QW + (qs + 1) * 512],
                            start=(ic == 0), stop=(ic == 3))
                    nc.vector.tensor_copy(q_t[u][:, qs * 512:(qs + 1) * 512],
                                          ps[:])
                return run

            def kproj(u, s, w):
                kt = kts[u]

                def run():
                    ps = psA.tile([128, 512], F32, tag="proj", name="ps")
                    for ic in range(4):
                        nc.tensor.matmul(ps[:, :w],
                                         wk[:, ic * 128:(ic + 1) * 128],
                                         xk[u][:, ic * kt + s:ic * kt + s + w],
                                         start=(ic == 0), stop=(ic == 3))
                    nc.vector.tensor_copy(k_t[u][:, s:s + w], ps[:, :w])
                return run

            def vproj(u, g):
                # one group = up to 4 key-chunks through a [128, 4, 128] PSUM
                # tile, landed with two strided 3D casts per head
                kt = kts[u]
                kcs = list(range(g * 4, min(NCH[u], g * 4 + 4)))
                nfull = sum(1 for kc in kcs if vls[u] - kc * 128 >= 128)

                def run():
                    ps = psA.tile([128, 4, 128], F32, tag="proj", name="ps")
                    for j, kc in enumerate(kcs):
                        for ic in range(4):
                            nc.tensor.matmul(
                                ps[:, j, :],
                                xv[u][:, ic * kt + kc * 128:
                                      ic * kt + (kc + 1) * 128],
                                wv[:, ic * 128:(ic + 1) * 128],
                                start=(ic == 0), stop=(ic == 3))
                    for h in range(2):
                        if nfull:
                            nc.vector.tensor_copy(
                                v_sb[u][:, kcs[0]:kcs[0] + nfull,
                                        h * 65:h * 65 + 64],
                                ps[:, 0:nfull, h * 64:(h + 1) * 64])
                    for j, kc in enumerate(kcs[nfull:], start=nfull):
                        nv = vls[u] - kc * 128
                        for h in range(2):
                            nc.vector.tensor_copy(
                                v_sb[u][0:nv, kc, h * 65:h * 65 + 64],
                                ps[0:nv, j, h * 64:(h + 1) * 64])
                return run

            def oproj(u, ot, qs):
                def run():
                    ps = psA.tile([128, 512], F32, tag="proj", name="ps")
                    nc.tensor.matmul(ps[:], wo[:, ot * 128:(ot + 1) * 128],
                                     o_pair[u][:, qs * 512:(qs + 1) * 512],
                                     start=True, stop=True)
                    y_sb = cb.tile([128, 512], BF16, tag="y", bufs=2,
                                   name="y_sb")
                    nc.vector.tensor_copy(y_sb[:], ps[:])
                    nc.sync.dma_start(
                        y_d[ot * 128:(ot + 1) * 128,
                            u * QW + qs * 512:u * QW + (qs + 1) * 512],
                        y_sb[:])
                return run

            def proj_tasks(u):
                t = []
                for s in range(0, kts[u], 512):
                    t.append(("proj", u, kproj(u, s, min(512, kts[u] - s))))
                for g in range((NCH[u] + 3) // 4):
                    t.append(("proj", u, vproj(u, g)))
                t += [("proj", u, qproj(u, 0)), ("proj", u, qproj(u, 1))]
                return t

            o_pair = {}

            # ---- flat attention pipeline over (unit, ip, kt) steps ----
            fillers = deque()
            fillers.extend(proj_tasks(1))

            for f in proj_tasks(0):   # unit 0 projections inline
                f[2]()

            steps = [(u, ip, kt) for u in range(4) for ip in range(2)
                     for kt in range(NCH[u])]

            def scores(u, ip, kt):
                s_ps = psS.tile([128, 1024], F32, tag="s", name="s_ps")
                for hf in range(2):
                    nc.tensor.matmul(
                        s_ps[:, hf * 512:(hf + 1) * 512],
                        k_t[u][hf * 64:(hf + 1) * 64, kt * 128:(kt + 1) * 128],
                        q_t[u][hf * 64:(hf + 1) * 64,
                               ip * 512:(ip + 1) * 512],
                        start=True, stop=True)
                p_sb = cb.tile([128, 1024], BF16, tag="p", bufs=4, name="p_sb")
                nc.scalar.activation(p_sb[:], s_ps[:], EXP, scale=1.0)
                return p_sb

            acc = {}

            def attnv(u, ip, kt, p_sb):
                if kt == 0:
                    acc["oA"] = psO.tile([65, 512], F32, tag="oA", name="oA")
                    acc["oB"] = psO.tile([65, 512], F32, tag="oB", name="oB")
                for hf, o_ps in enumerate((acc["oA"], acc["oB"])):
                    nc.tensor.matmul(
                        o_ps[:], v_sb[u][:, kt, hf * 65:hf * 65 + 65],
                        p_sb[:, hf * 512:(hf + 1) * 512],
                        start=(kt == 0), stop=(kt == NCH[u] - 1))

            def normalize(u, ip):
                if u not in o_pair:
                    o_pair[u] = cb.tile([128, QW], BF16, tag="o", bufs=2,
                                        name="o_pair")
                cols = slice(ip * 512, (ip + 1) * 512)
                for hf, o_ps in enumerate((acc["oA"], acc["oB"])):
                    dn = cb.tile([65, 512], F32R, tag="dn", bufs=2, name="dn")
                    nc.vector.tensor_copy(dn[64:65, :], o_ps[64:65, :])
                    bc_ps = psA.tile([64, 512], F32, tag="proj", name="bc_ps")
                    nc.tensor.matmul(bc_ps[:], ones_sb[64:65, :],
                                     dn[64:65, :], start=True, stop=True)
                    inv_sb = cb.tile([64, 512], F32, tag="invb", bufs=2,
                                     name="inv_sb")
                    nc.vector.reciprocal_approx_fast(inv_sb[:], bc_ps[:])
                    if hf == 0:
                        nc.vector.tensor_mul(o_pair[u][0:64, cols],
                                             o_ps[0:64, :], inv_sb[:])
                    else:
                        # DVE lanes can't cross partitions: normalize into a
                        # scratch tile, DMA-hop to partitions 64-127
                        o_tmp = cb.tile([64, 512], BF16, tag="otmp", bufs=2,
                                        name="o_tmp")
                        nc.vector.tensor_mul(o_tmp[:], o_ps[0:64, :],
                                             inv_sb[:])
                        nc.sync.dma_start(o_pair[u][64:128, cols], o_tmp[:])

            prev = None
            for step in steps:
                u, ip, kt = step
                if ip == 0 and kt == 0 and u > 0:
                    # everything feeding unit u must precede its scores on
                    # the PE stream (in-order queue => would deadlock after)
                    while any(t[0] == "proj" and t[1] <= u for t in fillers):
                        fillers.popleft()[2]()
                p = scores(u, ip, kt)
                if prev is not None:
                    pu, pip, pkt = prev[0]
                    attnv(pu, pip, pkt, prev[1])
                    if pkt == NCH[pu] - 1:
                        normalize(pu, pip)
                        # the q-half's output projection unblocks as soon as
                        # its own normalize is done -- don't wait for ip=1
                        for ot in range(4):
                            fillers.append(("oproj", pu, oproj(pu, ot, pip)))
                        if pip == 1 and pu + 2 <= 3:
                            fillers.extend(proj_tasks(pu + 2))
                if fillers:
                    fillers.popleft()[2]()
                extra = 2 if u == 3 else 1
                while extra and fillers and fillers[0][0] == "oproj":
                    fillers.popleft()[2]()
                    extra -= 1
                prev = (step, p)
            pu, pip, pkt = prev[0]
            attnv(pu, pip, pkt, prev[1])
            normalize(pu, pip)
            while fillers:
                fillers.popleft()[2]()
            for ot in range(4):
                oproj(pu, ot, pip)()

    nc.compile()
    return nc


def ceil128(v):
    return int(min(KSEQ, max(128, ((int(v) + 127) // 128) * 128)))


def plan(valid_lens):
    vl = np.asarray(valid_lens).astype(np.int64)
    asc = sorted(range(B), key=lambda b: (ceil128(vl[b]), b))
    # smallest unit last (short output tail), rest ascending (fast start)
    order = [asc[1], asc[2], asc[3], asc[0]]
    kts = tuple(ceil128(vl[b]) for b in order)
    vls = tuple(int(min(vl[b], kts[i])) for i, b in enumerate(order))
    return order, kts, vls


def _pack4(x):
    """[512, N] -> [128, 4N] with the four 128-row blocks side by side."""
    n = x.shape[1]
    return x.reshape(4, 128, n).transpose(1, 0, 2).reshape(128, 4 * n)


def make_in_maps(queries, keys, values, valid_lens, W_q, W_k, W_v, W_o,
                 order, kts):
    bf = ml_dtypes.bfloat16
    queries = np.asarray(queries, np.float32)
    keys = np.asarray(keys, np.float32)
    values = np.asarray(values, np.float32)
    W_q = np.asarray(W_q, np.float32)
    W_k = np.asarray(W_k, np.float32)
    W_v = np.asarray(W_v, np.float32)
    W_o = np.asarray(W_o, np.float32)
    in_maps = []
    for c in range(N_CORES):
        hp, qp = c % 4, c // 4
        fsl = slice(hp * 128, (hp + 1) * 128)
        qsl = slice(qp * QW, (qp + 1) * QW)
        xq = np.concatenate([_pack4(queries[b, qsl, :].T) for b in order],
                            axis=1)
        xk = np.concatenate([_pack4(keys[b, :kts[i], :].T)
                             for i, b in enumerate(order)], axis=1)
        xv = np.concatenate([_pack4(values[b, :kts[i], :].T)
                             for i, b in enumerate(order)], axis=1)
        in_maps.append({
            "xq_t": np.ascontiguousarray(xq).astype(bf),
            "xk_t": np.ascontiguousarray(xk).astype(bf),
            "xv_t": np.ascontiguousarray(xv).astype(bf),
            "wq_t": np.ascontiguousarray(_pack4((W_q[fsl, :] / 8.0).T)).astype(bf),
            "wk_t": np.ascontiguousarray(_pack4(W_k[fsl, :].T)).astype(bf),
            "wv_t": np.ascontiguousarray(_pack4(W_v[fsl, :].T)).astype(bf),
            "wo_t": np.ascontiguousarray(W_o[:, fsl].T).astype(bf),
        })
    return in_maps


def assemble(results, order):
    out = np.empty((B, Q, D), np.float32)
    for qp in range(2):
        for i, b in enumerate(order):
            acc = np.zeros((D, QW), np.float32)
            for hp in range(4):
                acc += np.asarray(results[qp * 4 + hp]["y_t"]
                                  [:, i * QW:(i + 1) * QW], np.float32)
            out[b, qp * QW:(qp + 1) * QW, :] = acc.T
    return out


def kernel(queries, keys, values, valid_lens, W_q, W_k, W_v, W_o):
    order, kts, vls = plan(valid_lens)
    nc = build_nc(kts, vls)
    in_maps = make_in_maps(queries, keys, values, valid_lens,
                           W_q, W_k, W_v, W_o, order, kts)
    res = run_bass_kernel_spmd(nc, in_maps, list(range(N_CORES))).results
    return assemble(res, order)


# revision 16
# speedup vs baseline: 1.6318x; 1.0785x over previous
"""Multi-head attention (B=4, Q=K=2048, D=512, H=8) on 8 TRN2 NeuronCores.

Sharding: every core runs the SAME program but a different (head-pair, q-half)
of every batch: core c owns heads {2*(c%4), 2*(c%4)+1} and query window
[1024*(c//4), 1024*(c//4)+1024) of ALL four batches.  Each batch is truncated
to its OWN KT_b = ceil128(valid_len[b]) -- key positions beyond valid_len have
softmax weight exactly 0, so per-batch truncation is exact and cuts total
attention work from 4*max(KT) to sum(KT).  Every core then processes exactly
sum_b KT_b/128 key-chunks: perfectly balanced by construction.

Device-side choices:
  * Activations transposed ([feature, seq]); matmuls contract the partition dim.
  * Scores computed transposed (S_T[k, q] = K_h @ Q_h^T).  The two heads of a
    core's pair sit on partitions 0-63 / 64-127 of shared q_t/k_t tiles, so
    their C=64 score matmuls land on disjoint PE row-groups (auto
    tile_position (0,0)/(64,0)) and run CONCURRENTLY in the array -- 2x score
    throughput vs. sequential heads.
  * No mask and no exp bias: chunks are either fully valid or the final
    partial chunk, whose invalid key rows are zeroed in v_sb (values AND the
    interleaved ones column), removing them from both the attnV numerator and
    the softmax denominator.  One exp covers both heads' score tiles
    ([128, 1024] PSUM spanning the pair's two banks).
  * Ones-column interleaved into v gives the softmax denominator for free
    (row 64 of each head's [65, 512] attnV accumulator).
  * All inputs arrive pre-packed host-side as [128, N] panels so each tensor
    is ONE large DMA descriptor (16 input DMAs total) -- the sync engine's
    per-descriptor issue cost otherwise starves the front of the kernel.
  * v_sb is a 3D [128, NCH, 130] tile per unit: V-proj runs 4 key-chunks per
    PSUM tile and lands them with two strided 3D casts instead of 8 small
    copies; ones columns are memset once per unit in the prologue.
  * bf16 matmul pipeline with fp32 PSUM; softmax/normalization fp32.
  * Partial-output projection per (core, batch); host sums the 4 head-pair
    partials per (batch, q-half).  Partials in bf16 to halve output DMA.
  * Projections of later units and output-projections of earlier units are
    emitted as filler tasks inside the attention loop so the PE never idles
    while the ACT engine (the attention-phase bottleneck) chews exps.
  * Units run in ascending-KT order rotated so the smallest unit comes last:
    the first unit's DMA is small (fast start) and the last unit's
    output-projection tail is short.
"""

import functools
from collections import deque

import ml_dtypes
import numpy as np

import concourse.bacc as bacc
import concourse.bass as bass
import concourse.mybir as mybir
from concourse import tile
from concourse.bass_utils import run_bass_kernel_spmd

F32 = mybir.dt.float32
F32R = mybir.dt.float32r
BF16 = mybir.dt.bfloat16

B, Q, KSEQ, D, H = 4, 2048, 2048, 512, 8
DH = D // H          # 64   head dim
QW = 1024            # per-core query window
N_CORES = 8
EXP = mybir.ActivationFunctionType.Exp


@functools.lru_cache(maxsize=4)
def build_nc(kts, vls):
    """One SPMD program; kts/vls are the per-unit (execution-ordered)
    key lengths / valid lens of the 4 batches."""
    assert all(kt % 128 == 0 and 128 <= kt <= KSEQ for kt in kts)
    NCH = [kt // 128 for kt in kts]
    KOFF = np.concatenate([[0], np.cumsum([4 * k for k in kts])]).tolist()
    SK4 = KOFF[-1]

    nc = bacc.Bacc("TRN2", target_bir_lowering=False, debug=False,
                   num_devices=N_CORES)

    def din(name, shape, dt=BF16):
        return nc.dram_tensor(name, shape, dt, kind="ExternalInput").ap()

    xq_d = din("xq_t", [128, 16 * QW])
    xk_d = din("xk_t", [128, SK4])
    xv_d = din("xv_t", [128, SK4])
    wq_d = din("wq_t", [128, D])
    wk_d = din("wk_t", [128, D])
    wv_d = din("wv_t", [128, D])
    wo_d = din("wo_t", [128, D])
    y_d = nc.dram_tensor("y_t", [D, 4 * QW], BF16, kind="ExternalOutput").ap()

    with tile.TileContext(nc) as tc:
        with (
            nc.allow_low_precision(reason="bf16 matmul operands"),
            tc.tile_pool(name="persist", bufs=1) as pp,
            tc.tile_pool(name="cbuf", bufs=1) as cb,
            # 8 PSUM banks: psS 2x[128,1024] score tiles (pair x 512q),
            # psO oA+oB [65,512] attnV accumulators, psA 2x[128,512]
            # projections / denominator broadcast.
            tc.tile_pool(name="psS", bufs=2, space=bass.MemorySpace.PSUM) as psS,
            tc.tile_pool(name="psO", bufs=1, space=bass.MemorySpace.PSUM) as psO,
            tc.tile_pool(name="psA", bufs=2, space=bass.MemorySpace.PSUM) as psA,
        ):
            # ---- persistent tiles ----
            wq = pp.tile([128, D], BF16, tag="wq", name="wq")
            wk = pp.tile([128, D], BF16, tag="wk", name="wk")
            wv = pp.tile([128, D], BF16, tag="wv", name="wv")
            wo = pp.tile([128, D], BF16, tag="wo", name="wo")
            onescr = pp.tile([128, DH], F32, tag="onescr", name="onescr")
            ones_sb = pp.tile([65, DH], F32R, tag="ones", name="ones_sb")
            actwarm = pp.tile([1, 1], F32, tag="actwarm", name="actwarm")

            xq = [pp.tile([128, 4 * QW], BF16, tag=f"xq{u}", name=f"xq{u}")
                  for u in range(4)]
            xk = [pp.tile([128, 4 * kts[u]], BF16, tag=f"xk{u}", name=f"xk{u}")
                  for u in range(4)]
            xv = [pp.tile([128, 4 * kts[u]], BF16, tag=f"xv{u}", name=f"xv{u}")
                  for u in range(4)]
            q_t = [pp.tile([128, QW], BF16, tag=f"q_t{u}", name=f"q_t{u}")
                   for u in range(4)]
            k_t = [pp.tile([128, kts[u]], BF16, tag=f"k_t{u}", name=f"k_t{u}")
                   for u in range(4)]
            v_sb = [pp.tile([128, NCH[u], 130], BF16, tag=f"v{u}",
                            name=f"v{u}") for u in range(4)]

            # ---- DMAs on TWO hardware queues (sync + scalar HWDGE) so the
            # k/v stream and the weights/q stream transfer in parallel ----
            nc.sync.dma_start(wk[:], wk_d[:])
            nc.scalar.dma_start(wq[:], wq_d[:])
            # pull the ACT exp table load into the initial DMA wait
            nc.vector.memset(onescr[:], 1.0)
            nc.scalar.activation(actwarm[:], onescr[0:1, 0:1], EXP)
            nc.vector.tensor_copy(ones_sb[64:65, :], onescr[64:65, :])
            # warm the PE HAM clock gate during the initial DMA wait: ~4us of
            # dummy matmuls flips the clock to 2.4 GHz before the first
            # projection, halving the cold-start cost of unit 0
            warm_ps = psA.tile([64, 64], F32, tag="proj", name="warm_ps")
            for _ in range(56):
                nc.tensor.matmul(warm_ps[:], onescr[0:64, :],
                                 onescr[0:64, :], start=True, stop=True)
            # prologue memsets: ones columns for every unit's v_sb; zero the
            # final partial chunk first so its invalid rows stay zero.
            for u in range(4):
                nv = vls[u] - (NCH[u] - 1) * 128
                if nv < 128:
                    if NCH[u] > 1:
                        nc.vector.memset(v_sb[u][:, 0:NCH[u] - 1, 64::65], 1.0)
                    nc.vector.memset(v_sb[u][:, NCH[u] - 1, :], 0.0)
                    nc.vector.memset(v_sb[u][0:nv, NCH[u] - 1, 64::65], 1.0)
                else:
                    nc.vector.memset(v_sb[u][:, :, 64::65], 1.0)
            nc.sync.dma_start(xk[0][:], xk_d[:, KOFF[0]:KOFF[1]])
            nc.scalar.dma_start(wv[:], wv_d[:])
            nc.sync.dma_start(xv[0][:], xv_d[:, KOFF[0]:KOFF[1]])
            nc.scalar.dma_start(xq[0][:], xq_d[:, 0:4 * QW])
            nc.scalar.dma_start(wo[:], wo_d[:])
            for u in range(1, 4):
                nc.sync.dma_start(xk[u][:], xk_d[:, KOFF[u]:KOFF[u + 1]])
                nc.sync.dma_start(xv[u][:], xv_d[:, KOFF[u]:KOFF[u + 1]])
                nc.scalar.dma_start(xq[u][:],
                                    xq_d[:, u * 4 * QW:(u + 1) * 4 * QW])

            # ---- projection / output-projection task factories ----
            def qproj(u, qs):
                def run():
                    ps = psA.tile([128, 512], F32, tag="proj", name="ps")
                    for ic in range(4):
                        nc.tensor.matmul(
                            ps[:], wq[:, ic * 128:(ic + 1) * 128],
                            xq[u][:, ic * QW + qs * 512:
                                  ic * QW + (qs + 1) * 512],
                            start=(ic == 0), stop=(ic == 3))
                    nc.vector.tensor_copy(q_t[u][:, qs * 512:(qs + 1) * 512],
                                          ps[:])
                return run

            def kproj(u, s, w):
                kt = kts[u]

                def run():
                    ps = psA.tile([128, 512], F32, tag="proj", name="ps")
                    for ic in range(4):
                        nc.tensor.matmul(ps[:, :w],
                                         wk[:, ic * 128:(ic + 1) * 128],
                                         xk[u][:, ic * kt + s:ic * kt + s + w],
                                         start=(ic == 0), stop=(ic == 3))
                    nc.vector.tensor_copy(k_t[u][:, s:s + w], ps[:, :w])
                return run

            def vproj(u, g):
                # one group = up to 4 key-chunks through a [128, 4, 128] PSUM
                # tile, landed with two strided 3D casts per head
                kt = kts[u]
                kcs = list(range(g * 4, min(NCH[u], g * 4 + 4)))
                nfull = sum(1 for kc in kcs if vls[u] - kc * 128 >= 128)

                def run():
                    ps = psA.tile([128, 4, 128], F32, tag="proj", name="ps")
                    for j, kc in enumerate(kcs):
                        for ic in range(4):
                            nc.tensor.matmul(
                                ps[:, j, :],
                                xv[u][:, ic * kt + kc * 128:
                                      ic * kt + (kc + 1) * 128],
                                wv[:, ic * 128:(ic + 1) * 128],
                                start=(ic == 0), stop=(ic == 3))
                    for h in range(2):
                        if nfull:
                            nc.vector.tensor_copy(
                                v_sb[u][:, kcs[0]:kcs[0] + nfull,
                                        h * 65:h * 65 + 64],
                                ps[:, 0:nfull, h * 64:(h + 1) * 64])
                    for j, kc in enumerate(kcs[nfull:], start=nfull):
                        nv = vls[u] - kc * 128
                        for h in range(2):
                            nc.vector.tensor_copy(
                                v_sb[u][0:nv, kc, h * 65:h * 65 + 64],
                                ps[0:nv, j, h * 64:(h + 1) * 64])
                return run

            def oproj(u, ot, qs):
                def run():
                    ps = psA.tile([128, 512], F32, tag="proj", name="ps")
                    nc.tensor.matmul(ps[:], wo[:, ot * 128:(ot + 1) * 128],
                                     o_pair[u][:, qs * 512:(qs + 1) * 512],
                                     start=True, stop=True)
                    y_sb = cb.tile([128, 512], BF16, tag="y", bufs=2,
                                   name="y_sb")
                    nc.vector.tensor_copy(y_sb[:], ps[:])
                    nc.sync.dma_start(
                        y_d[ot * 128:(ot + 1) * 128,
                            u * QW + qs * 512:u * QW + (qs + 1) * 512],
                        y_sb[:])
                return run

            def proj_tasks(u):
                t = []
                for s in range(0, kts[u], 512):
                    t.append(("proj", u, kproj(u, s, min(512, kts[u] - s))))
                for g in range((NCH[u] + 3) // 4):
                    t.append(("proj", u, vproj(u, g)))
                t += [("proj", u, qproj(u, 0)), ("proj", u, qproj(u, 1))]
                return t

            o_pair = {}

            # ---- flat attention pipeline over (unit, ip, kt) steps ----
            fillers = deque()
            # unit 0: only what attention ip=0 needs goes inline; the second
            # q-half projection becomes the first filler
            t0 = proj_tasks(0)
            for f in t0[:-1]:
                f[2]()
            fillers.append(t0[-1])
            fillers.extend(proj_tasks(1))

            steps = [(u, ip, kt) for u in range(4) for ip in range(2)
                     for kt in range(NCH[u])]

            def scores(u, ip, kt):
                s_ps = psS.tile([128, 1024], F32, tag="s", name="s_ps")
                for hf in range(2):
                    nc.tensor.matmul(
                        s_ps[:, hf * 512:(hf + 1) * 512],
                        k_t[u][hf * 64:(hf + 1) * 64, kt * 128:(kt + 1) * 128],
                        q_t[u][hf * 64:(hf + 1) * 64,
                               ip * 512:(ip + 1) * 512],
                        start=True, stop=True)
                p_sb = cb.tile([128, 1024], BF16, tag="p", bufs=4, name="p_sb")
                nc.scalar.activation(p_sb[:], s_ps[:], EXP, scale=1.0)
                return p_sb

            acc = {}

            def attnv(u, ip, kt, p_sb):
                if kt == 0:
                    acc["oA"] = psO.tile([65, 512], F32, tag="oA", name="oA")
                    acc["oB"] = psO.tile([65, 512], F32, tag="oB", name="oB")
                for hf, o_ps in enumerate((acc["oA"], acc["oB"])):
                    nc.tensor.matmul(
                        o_ps[:], v_sb[u][:, kt, hf * 65:hf * 65 + 65],
                        p_sb[:, hf * 512:(hf + 1) * 512],
                        start=(kt == 0), stop=(kt == NCH[u] - 1))

            def normalize(u, ip):
                if u not in o_pair:
                    o_pair[u] = cb.tile([128, QW], BF16, tag="o", bufs=2,
                                        name="o_pair")
                cols = slice(ip * 512, (ip + 1) * 512)
                for hf, o_ps in enumerate((acc["oA"], acc["oB"])):
                    dn = cb.tile([65, 512], F32R, tag="dn", bufs=2, name="dn")
                    nc.vector.tensor_copy(dn[64:65, :], o_ps[64:65, :])
                    bc_ps = psA.tile([64, 512], F32, tag="proj", name="bc_ps")
                    nc.tensor.matmul(bc_ps[:], ones_sb[64:65, :],
                                     dn[64:65, :], start=True, stop=True)
                    inv_sb = cb.tile([64, 512], F32, tag="invb", bufs=2,
                                     name="inv_sb")
                    nc.vector.reciprocal_approx_fast(inv_sb[:], bc_ps[:])
                    if hf == 0:
                        nc.vector.tensor_mul(o_pair[u][0:64, cols],
                                             o_ps[0:64, :], inv_sb[:])
                    else:
                        # DVE lanes can't cross partitions: normalize into a
                        # scratch tile, DMA-hop to partitions 64-127
                        o_tmp = cb.tile([64, 512], BF16, tag="otmp", bufs=2,
                                        name="o_tmp")
                        nc.vector.tensor_mul(o_tmp[:], o_ps[0:64, :],
                                             inv_sb[:])
                        nc.sync.dma_start(o_pair[u][64:128, cols], o_tmp[:])

            prev = None
            for step in steps:
                u, ip, kt = step
                if ip == 0 and kt == 0 and u > 0:
                    # everything feeding unit u must precede its scores on
                    # the PE stream (in-order queue => would deadlock after)
                    while any(t[0] == "proj" and t[1] <= u for t in fillers):
                        fillers.popleft()[2]()
                p = scores(u, ip, kt)
                if prev is not None:
                    pu, pip, pkt = prev[0]
                    attnv(pu, pip, pkt, prev[1])
                    if pkt == NCH[pu] - 1:
                        normalize(pu, pip)
                        # the q-half's output projection unblocks as soon as
                        # its own normalize is done -- don't wait for ip=1
                        for ot in range(4):
                            fillers.append(("oproj", pu, oproj(pu, ot, pip)))
                        if pip == 1 and pu + 2 <= 3:
                            # interleave the new unit's projections among the
                            # queued oproj tasks so each attention step can
                            # retire one of each kind
                            from itertools import zip_longest
                            old = list(fillers)
                            fillers.clear()
                            for pair in zip_longest(proj_tasks(pu + 2), old):
                                for t in pair:
                                    if t is not None:
                                        fillers.append(t)
                if fillers:
                    fillers.popleft()[2]()
                extra = 3 if u == 3 else 1
                while extra and fillers and fillers[0][0] == "oproj":
                    fillers.popleft()[2]()
                    extra -= 1
                prev = (step, p)
            pu, pip, pkt = prev[0]
            attnv(pu, pip, pkt, prev[1])
            normalize(pu, pip)
            while fillers:
                fillers.popleft()[2]()
            for ot in range(4):
                oproj(pu, ot, pip)()

    nc.compile()
    return nc


def ceil128(v):
    return int(min(KSEQ, max(128, ((int(v) + 127) // 128) * 128)))


def plan(valid_lens):
    vl = np.asarray(valid_lens).astype(np.int64)
    asc = sorted(range(B), key=lambda b: (ceil128(vl[b]), b))
    # smallest first (fast DMA-gated start), largest third (its long
    # attention phase absorbs earlier units' output projections), mid-size
    # last (enough steps to drain the giant's output work before the tail)
    order = [asc[0], asc[2], asc[3], asc[1]]
    kts = tuple(ceil128(vl[b]) for b in order)
    vls = tuple(int(min(vl[b], kts[i])) for i, b in enumerate(order))
    return order, kts, vls


def _pack4(x):
    """[512, N] -> [128, 4N] with the four 128-row blocks side by side."""
    n = x.shape[1]
    return x.reshape(4, 128, n).transpose(1, 0, 2).reshape(128, 4 * n)


def make_in_maps(queries, keys, values, valid_lens, W_q, W_k, W_v, W_o,
                 order, kts):
    bf = ml_dtypes.bfloat16
    queries = np.asarray(queries, np.float32)
    keys = np.asarray(keys, np.float32)
    values = np.asarray(values, np.float32)
    W_q = np.asarray(W_q, np.float32)
    W_k = np.asarray(W_k, np.float32)
    W_v = np.asarray(W_v, np.float32)
    W_o = np.asarray(W_o, np.float32)
    in_maps = []
    for c in range(N_CORES):
        hp, qp = c % 4, c // 4
        fsl = slice(hp * 128, (hp + 1) * 128)
        qsl = slice(qp * QW, (qp + 1) * QW)
        xq = np.concatenate([_pack4(queries[b, qsl, :].T) for b in order],
                            axis=1)
        xk = np.concatenate([_pack4(keys[b, :kts[i], :].T)
                             for i, b in enumerate(order)], axis=1)
        xv = np.concatenate([_pack4(values[b, :kts[i], :].T)
                             for i, b in enumerate(order)], axis=1)
        in_maps.append({
            "xq_t": np.ascontiguousarray(xq).astype(bf),
            "xk_t": np.ascontiguousarray(xk).astype(bf),
            "xv_t": np.ascontiguousarray(xv).astype(bf),
            "wq_t": np.ascontiguousarray(_pack4((W_q[fsl, :] / 8.0).T)).astype(bf),
            "wk_t": np.ascontiguousarray(_pack4(W_k[fsl, :].T)).astype(bf),
            "wv_t": np.ascontiguousarray(_pack4(W_v[fsl, :].T)).astype(bf),
            "wo_t": np.ascontiguousarray(W_o[:, fsl].T).astype(bf),
        })
    return in_maps


def assemble(results, order):
    out = np.empty((B, Q, D), np.float32)
    for qp in range(2):
        for i, b in enumerate(order):
            acc = np.zeros((D, QW), np.float32)
            for hp in range(4):
                acc += np.asarray(results[qp * 4 + hp]["y_t"]
                                  [:, i * QW:(i + 1) * QW], np.float32)
            out[b, qp * QW:(qp + 1) * QW, :] = acc.T
    return out


def kernel(queries, keys, values, valid_lens, W_q, W_k, W_v, W_o):
    order, kts, vls = plan(valid_lens)
    nc = build_nc(kts, vls)
    in_maps = make_in_maps(queries, keys, values, valid_lens,
                           W_q, W_k, W_v, W_o, order, kts)
    res = run_bass_kernel_spmd(nc, in_maps, list(range(N_CORES))).results
    return assemble(res, order)


# revision 20
# speedup vs baseline: 1.6806x; 1.0299x over previous
"""Multi-head attention (B=4, Q=K=2048, D=512, H=8) on 8 TRN2 NeuronCores.

Sharding: every core runs the SAME program but a different (head-pair, q-half)
of every batch: core c owns heads {2*(c%4), 2*(c%4)+1} and query window
[1024*(c//4), 1024*(c//4)+1024) of ALL four batches.  Each batch is truncated
to its OWN KT_b = ceil128(valid_len[b]) -- key positions beyond valid_len have
softmax weight exactly 0, so per-batch truncation is exact and cuts total
attention work from 4*max(KT) to sum(KT).  Every core then processes exactly
sum_b KT_b/128 key-chunks: perfectly balanced by construction.

Device-side choices:
  * Activations transposed ([feature, seq]); matmuls contract the partition dim.
  * Scores computed transposed (S_T[k, q] = K_h @ Q_h^T).  The two heads of a
    core's pair sit on partitions 0-63 / 64-127 of shared q_t/k_t tiles, so
    their C=64 score matmuls land on disjoint PE row-groups (auto
    tile_position (0,0)/(64,0)) and run CONCURRENTLY in the array -- 2x score
    throughput vs. sequential heads.
  * No mask and no exp bias: chunks are either fully valid or the final
    partial chunk, whose invalid key rows are zeroed in v_sb (values AND the
    interleaved ones column), removing them from both the attnV numerator and
    the softmax denominator.  One exp covers both heads' score tiles
    ([128, 1024] PSUM spanning the pair's two banks).
  * Ones-column interleaved into v gives the softmax denominator for free
    (row 64 of each head's [65, 512] attnV accumulator).
  * All inputs arrive pre-packed host-side as [128, N] panels so each tensor
    is ONE large DMA descriptor (16 input DMAs total) -- the sync engine's
    per-descriptor issue cost otherwise starves the front of the kernel.
  * v_sb is a 3D [128, NCH, 130] tile per unit: V-proj runs 4 key-chunks per
    PSUM tile and lands them with two strided 3D casts instead of 8 small
    copies; ones columns are memset once per unit in the prologue.
  * bf16 matmul pipeline with fp32 PSUM; softmax/normalization fp32.
  * Partial-output projection per (core, batch); host sums the 4 head-pair
    partials per (batch, q-half).  Partials in bf16 to halve output DMA.
  * Projections of later units and output-projections of earlier units are
    emitted as filler tasks inside the attention loop so the PE never idles
    while the ACT engine (the attention-phase bottleneck) chews exps.
  * Units run in ascending-KT order rotated so the smallest unit comes last:
    the first unit's DMA is small (fast start) and the last unit's
    output-projection tail is short.
"""

import functools
from collections import deque

import ml_dtypes
import numpy as np

import concourse.bacc as bacc
import concourse.bass as bass
import concourse.mybir as mybir
from concourse import tile
from concourse.bass_utils import run_bass_kernel_spmd

F32 = mybir.dt.float32
F32R = mybir.dt.float32r
BF16 = mybir.dt.bfloat16

B, Q, KSEQ, D, H = 4, 2048, 2048, 512, 8
DH = D // H          # 64   head dim
QW = 1024            # per-core query window
N_CORES = 8
EXP = mybir.ActivationFunctionType.Exp


@functools.lru_cache(maxsize=4)
def build_nc(kts, vls):
    """One SPMD program; kts/vls are the per-unit (execution-ordered)
    key lengths / valid lens of the 4 batches."""
    assert all(kt % 128 == 0 and 128 <= kt <= KSEQ for kt in kts)
    NCH = [kt // 128 for kt in kts]
    KOFF = np.concatenate([[0], np.cumsum([4 * k for k in kts])]).tolist()
    SK4 = KOFF[-1]

    nc = bacc.Bacc("TRN2", target_bir_lowering=False, debug=False,
                   num_devices=N_CORES)

    def din(name, shape, dt=BF16):
        return nc.dram_tensor(name, shape, dt, kind="ExternalInput").ap()

    xq_d = din("xq_t", [128, 16 * QW])
    xk_d = din("xk_t", [128, SK4])
    xv_d = din("xv_t", [128, SK4])
    wq_d = din("wq_t", [128, D])
    wk_d = din("wk_t", [128, D])
    wv_d = din("wv_t", [128, D])
    wo_d = din("wo_t", [128, D])
    y_d = nc.dram_tensor("y_t", [D, 4 * QW], BF16, kind="ExternalOutput").ap()

    with tile.TileContext(nc) as tc:
        with (
            nc.allow_low_precision(reason="bf16 matmul operands"),
            tc.tile_pool(name="persist", bufs=1) as pp,
            tc.tile_pool(name="cbuf", bufs=1) as cb,
            # 8 PSUM banks: psS 2x[128,1024] score tiles (pair x 512q),
            # psO oA+oB [65,512] attnV accumulators, psA 2x[128,512]
            # projections / denominator broadcast.
            tc.tile_pool(name="psS", bufs=2, space=bass.MemorySpace.PSUM) as psS,
            tc.tile_pool(name="psO", bufs=1, space=bass.MemorySpace.PSUM) as psO,
            tc.tile_pool(name="psA", bufs=2, space=bass.MemorySpace.PSUM) as psA,
        ):
            # ---- persistent tiles ----
            wq = pp.tile([128, D], BF16, tag="wq", name="wq")
            wk = pp.tile([128, D], BF16, tag="wk", name="wk")
            wv = pp.tile([128, D], BF16, tag="wv", name="wv")
            wo = pp.tile([128, D], BF16, tag="wo", name="wo")
            onescr = pp.tile([128, DH], F32, tag="onescr", name="onescr")
            ones_sb = pp.tile([65, DH], F32R, tag="ones", name="ones_sb")
            actwarm = pp.tile([1, 1], F32, tag="actwarm", name="actwarm")

            xq = [pp.tile([128, 4 * QW], BF16, tag=f"xq{u}", name=f"xq{u}")
                  for u in range(4)]
            xk = [pp.tile([128, 4 * kts[u]], BF16, tag=f"xk{u}", name=f"xk{u}")
                  for u in range(4)]
            xv = [pp.tile([128, 4 * kts[u]], BF16, tag=f"xv{u}", name=f"xv{u}")
                  for u in range(4)]
            q_t = [pp.tile([128, QW], BF16, tag=f"q_t{u}", name=f"q_t{u}")
                   for u in range(4)]
            k_t = [pp.tile([128, kts[u]], BF16, tag=f"k_t{u}", name=f"k_t{u}")
                   for u in range(4)]
            v_sb = [pp.tile([128, NCH[u], 130], BF16, tag=f"v{u}",
                            name=f"v{u}") for u in range(4)]

            # ---- DMAs on TWO hardware queues (sync + scalar HWDGE) so the
            # k/v stream and the weights/q stream transfer in parallel ----
            nc.sync.dma_start(wk[:], wk_d[:])
            nc.scalar.dma_start(wq[:], wq_d[:])
            # pull the ACT exp table load into the initial DMA wait
            nc.vector.memset(onescr[:], 1.0)
            nc.scalar.activation(actwarm[:], onescr[0:1, 0:1], EXP)
            nc.vector.tensor_copy(ones_sb[64:65, :], onescr[64:65, :])

            # prologue memsets: ones columns for every unit's v_sb; zero the
            # final partial chunk first so its invalid rows stay zero.
            for u in range(4):
                nv = vls[u] - (NCH[u] - 1) * 128
                if nv < 128:
                    if NCH[u] > 1:
                        nc.vector.memset(v_sb[u][:, 0:NCH[u] - 1, 64::65], 1.0)
                    nc.vector.memset(v_sb[u][:, NCH[u] - 1, :], 0.0)
                    nc.vector.memset(v_sb[u][0:nv, NCH[u] - 1, 64::65], 1.0)
                else:
                    nc.vector.memset(v_sb[u][:, :, 64::65], 1.0)
            nc.sync.dma_start(xk[0][:], xk_d[:, KOFF[0]:KOFF[1]])
            nc.scalar.dma_start(wv[:], wv_d[:])
            nc.sync.dma_start(xv[0][:], xv_d[:, KOFF[0]:KOFF[1]])
            nc.scalar.dma_start(xq[0][:], xq_d[:, 0:4 * QW])
            nc.scalar.dma_start(wo[:], wo_d[:])
            for u in range(1, 4):
                nc.sync.dma_start(xk[u][:], xk_d[:, KOFF[u]:KOFF[u + 1]])
                nc.sync.dma_start(xv[u][:], xv_d[:, KOFF[u]:KOFF[u + 1]])
                nc.scalar.dma_start(xq[u][:],
                                    xq_d[:, u * 4 * QW:(u + 1) * 4 * QW])

            # ---- projection / output-projection task factories ----
            def qproj(u, qs):
                def run():
                    ps = psA.tile([128, 512], F32, tag="proj", name="ps")
                    for ic in range(4):
                        nc.tensor.matmul(
                            ps[:], wq[:, ic * 128:(ic + 1) * 128],
                            xq[u][:, ic * QW + qs * 512:
                                  ic * QW + (qs + 1) * 512],
                            start=(ic == 0), stop=(ic == 3))
                    nc.vector.tensor_copy(q_t[u][:, qs * 512:(qs + 1) * 512],
                                          ps[:])
                return run

            def kproj(u, s, w):
                kt = kts[u]

                def run():
                    ps = psA.tile([128, 512], F32, tag="proj", name="ps")
                    for ic in range(4):
                        nc.tensor.matmul(ps[:, :w],
                                         wk[:, ic * 128:(ic + 1) * 128],
                                         xk[u][:, ic * kt + s:ic * kt + s + w],
                                         start=(ic == 0), stop=(ic == 3))
                    nc.vector.tensor_copy(k_t[u][:, s:s + w], ps[:, :w])
                return run

            def vproj(u, g):
                # one group = up to 4 key-chunks through a [128, 4, 128] PSUM
                # tile, landed with two strided 3D casts per head
                kt = kts[u]
                kcs = list(range(g * 4, min(NCH[u], g * 4 + 4)))
                nfull = sum(1 for kc in kcs if vls[u] - kc * 128 >= 128)

                def run():
                    ps = psA.tile([128, 4, 128], F32, tag="proj", name="ps")
                    for j, kc in enumerate(kcs):
                        for ic in range(4):
                            nc.tensor.matmul(
                                ps[:, j, :],
                                xv[u][:, ic * kt + kc * 128:
                                      ic * kt + (kc + 1) * 128],
                                wv[:, ic * 128:(ic + 1) * 128],
                                start=(ic == 0), stop=(ic == 3))
                    for h in range(2):
                        if nfull:
                            nc.vector.tensor_copy(
                                v_sb[u][:, kcs[0]:kcs[0] + nfull,
                                        h * 65:h * 65 + 64],
                                ps[:, 0:nfull, h * 64:(h + 1) * 64])
                    for j, kc in enumerate(kcs[nfull:], start=nfull):
                        nv = vls[u] - kc * 128
                        for h in range(2):
                            nc.vector.tensor_copy(
                                v_sb[u][0:nv, kc, h * 65:h * 65 + 64],
                                ps[0:nv, j, h * 64:(h + 1) * 64])
                return run

            def oproj(u, ot, qs):
                def run():
                    ps = psA.tile([128, 512], F32, tag="proj", name="ps")
                    nc.tensor.matmul(ps[:], wo[:, ot * 128:(ot + 1) * 128],
                                     o_pair[u][:, qs * 512:(qs + 1) * 512],
                                     start=True, stop=True)
                    y_sb = cb.tile([128, 512], BF16, tag="y", bufs=2,
                                   name="y_sb")
                    nc.vector.tensor_copy(y_sb[:], ps[:])
                    nc.sync.dma_start(
                        y_d[ot * 128:(ot + 1) * 128,
                            u * QW + qs * 512:u * QW + (qs + 1) * 512],
                        y_sb[:])
                return run

            def proj_tasks(u):
                # each task carries the first local attention step (ip*NCH+kt)
                # that consumes its output, enabling just-in-time draining
                t = []
                for s in range(0, kts[u], 512):
                    t.append(("proj", u, s // 128,
                              kproj(u, s, min(512, kts[u] - s))))
                for g in range((NCH[u] + 3) // 4):
                    t.append(("proj", u, g * 4, vproj(u, g)))
                t.append(("proj", u, 0, qproj(u, 0)))
                t.append(("proj", u, NCH[u], qproj(u, 1)))
                t.sort(key=lambda x: x[2])
                return t

            o_pair = {}

            # ---- flat attention pipeline over (unit, ip, kt) steps ----
            fillers = deque()
            # unit 0: only what the first attention steps need goes inline;
            # the rest becomes fillers drained just in time
            for f in proj_tasks(0):
                if f[2] == 0:
                    f[3]()
                else:
                    fillers.append(f)
            fillers.extend(proj_tasks(1))

            steps = [(u, ip, kt) for u in range(4) for ip in range(2)
                     for kt in range(NCH[u])]

            def scores(u, ip, kt):
                s_ps = psS.tile([128, 1024], F32, tag="s", name="s_ps")
                for hf in range(2):
                    nc.tensor.matmul(
                        s_ps[:, hf * 512:(hf + 1) * 512],
                        k_t[u][hf * 64:(hf + 1) * 64, kt * 128:(kt + 1) * 128],
                        q_t[u][hf * 64:(hf + 1) * 64,
                               ip * 512:(ip + 1) * 512],
                        start=True, stop=True)
                p_sb = cb.tile([128, 1024], BF16, tag="p", bufs=4, name="p_sb")
                nc.scalar.activation(p_sb[:], s_ps[:], EXP, scale=1.0)
                return p_sb

            acc = {}

            def attnv(u, ip, kt, p_sb):
                if kt == 0:
                    acc["oA"] = psO.tile([65, 512], F32, tag="oA", name="oA")
                    acc["oB"] = psO.tile([65, 512], F32, tag="oB", name="oB")
                for hf, o_ps in enumerate((acc["oA"], acc["oB"])):
                    nc.tensor.matmul(
                        o_ps[:], v_sb[u][:, kt, hf * 65:hf * 65 + 65],
                        p_sb[:, hf * 512:(hf + 1) * 512],
                        start=(kt == 0), stop=(kt == NCH[u] - 1))

            def normalize(u, ip):
                if u not in o_pair:
                    o_pair[u] = cb.tile([128, QW], BF16, tag="o", bufs=2,
                                        name="o_pair")
                cols = slice(ip * 512, (ip + 1) * 512)
                for hf, o_ps in enumerate((acc["oA"], acc["oB"])):
                    dn = cb.tile([65, 512], F32R, tag="dn", bufs=2, name="dn")
                    nc.vector.tensor_copy(dn[64:65, :], o_ps[64:65, :])
                    bc_ps = psA.tile([64, 512], F32, tag="proj", name="bc_ps")
                    nc.tensor.matmul(bc_ps[:], ones_sb[64:65, :],
                                     dn[64:65, :], start=True, stop=True)
                    inv_sb = cb.tile([64, 512], F32, tag="invb", bufs=2,
                                     name="inv_sb")
                    nc.vector.reciprocal_approx_fast(inv_sb[:], bc_ps[:])
                    if hf == 0:
                        nc.vector.tensor_mul(o_pair[u][0:64, cols],
                                             o_ps[0:64, :], inv_sb[:])
                    else:
                        # DVE lanes can't cross partitions: normalize into a
                        # scratch tile, DMA-hop to partitions 64-127
                        o_tmp = cb.tile([64, 512], BF16, tag="otmp", bufs=2,
                                        name="o_tmp")
                        nc.vector.tensor_mul(o_tmp[:], o_ps[0:64, :],
                                             inv_sb[:])
                        nc.sync.dma_start(o_pair[u][64:128, cols], o_tmp[:])

            prev = None
            for step in steps:
                u, ip, kt = step
                # just-in-time drain: run any projection task whose output an
                # imminent step consumes (in-order PE queue => emitting it
                # after its consumer would deadlock); leave the rest queued
                local = ip * NCH[u] + kt
                due = [t for t in fillers
                       if t[0] == "proj" and
                       (t[1] < u or (t[1] == u and t[2] <= local + 2))]
                if due:
                    rest = [t for t in fillers if t not in due]
                    fillers.clear()
                    fillers.extend(rest)
                    for t in due:
                        t[3]()
                p = scores(u, ip, kt)
                if prev is not None:
                    pu, pip, pkt = prev[0]
                    attnv(pu, pip, pkt, prev[1])
                    if pkt == NCH[pu] - 1:
                        normalize(pu, pip)
                        # the q-half's output projection unblocks as soon as
                        # its own normalize is done -- don't wait for ip=1
                        for ot in range(4):
                            fillers.append(("oproj", pu, 0,
                                            oproj(pu, ot, pip)))
                        if pip == 1 and pu + 2 <= 3:
                            # interleave the new unit's projections among the
                            # queued oproj tasks so each attention step can
                            # retire one of each kind
                            from itertools import zip_longest
                            old = list(fillers)
                            fillers.clear()
                            for pair in zip_longest(proj_tasks(pu + 2), old):
                                for t in pair:
                                    if t is not None:
                                        fillers.append(t)
                if fillers:
                    fillers.popleft()[3]()
                extra = 3 if u == 3 else 1
                while extra and fillers and fillers[0][0] == "oproj":
                    fillers.popleft()[3]()
                    extra -= 1
                prev = (step, p)
            pu, pip, pkt = prev[0]
            attnv(pu, pip, pkt, prev[1])
            normalize(pu, pip)
            while fillers:
                fillers.popleft()[3]()
            for ot in range(4):
                oproj(pu, ot, pip)()

    nc.compile()
    return nc


def ceil128(v):
    return int(min(KSEQ, max(128, ((int(v) + 127) // 128) * 128)))


def plan(valid_lens):
    vl = np.asarray(valid_lens).astype(np.int64)
    asc = sorted(range(B), key=lambda b: (ceil128(vl[b]), b))
    # smallest first (fast DMA-gated start), largest third (its long
    # attention phase absorbs earlier units' output projections), mid-size
    # last (enough steps to drain the giant's output work before the tail)
    order = [asc[0], asc[2], asc[3], asc[1]]
    kts = tuple(ceil128(vl[b]) for b in order)
    vls = tuple(int(min(vl[b], kts[i])) for i, b in enumerate(order))
    return order, kts, vls


def _pack4(x):
    """[512, N] -> [128, 4N] with the four 128-row blocks side by side."""
    n = x.shape[1]
    return x.reshape(4, 128, n).transpose(1, 0, 2).reshape(128, 4 * n)


def make_in_maps(queries, keys, values, valid_lens, W_q, W_k, W_v, W_o,
                 order, kts):
    bf = ml_dtypes.bfloat16
    queries = np.asarray(queries, np.float32)
    keys = np.asarray(keys, np.float32)
    values = np.asarray(values, np.float32)
    W_q = np.asarray(W_q, np.float32)
    W_k = np.asarray(W_k, np.float32)
    W_v = np.asarray(W_v, np.float32)
    W_o = np.asarray(W_o, np.float32)
    in_maps = []
    for c in range(N_CORES):
        hp, qp = c % 4, c // 4
        fsl = slice(hp * 128, (hp + 1) * 128)
        qsl = slice(qp * QW, (qp + 1) * QW)
        xq = np.concatenate([_pack4(queries[b, qsl, :].T) for b in order],
                            axis=1)
        xk = np.concatenate([_pack4(keys[b, :kts[i], :].T)
                             for i, b in enumerate(order)], axis=1)
        xv = np.concatenate([_pack4(values[b, :kts[i], :].T)
                             for i, b in enumerate(order)], axis=1)
        in_maps.append({
            "xq_t": np.ascontiguousarray(xq).astype(bf),
            "xk_t": np.ascontiguousarray(xk).astype(bf),
            "xv_t": np.ascontiguousarray(xv).astype(bf),
            "wq_t": np.ascontiguousarray(_pack4((W_q[fsl, :] / 8.0).T)).astype(bf),
            "wk_t": np.ascontiguousarray(_pack4(W_k[fsl, :].T)).astype(bf),
            "wv_t": np.ascontiguousarray(_pack4(W_v[fsl, :].T)).astype(bf),
            "wo_t": np.ascontiguousarray(W_o[:, fsl].T).astype(bf),
        })
    return in_maps


def assemble(results, order):
    out = np.empty((B, Q, D), np.float32)
    for qp in range(2):
        for i, b in enumerate(order):
            acc = np.zeros((D, QW), np.float32)
            for hp in range(4):
                acc += np.asarray(results[qp * 4 + hp]["y_t"]
                                  [:, i * QW:(i + 1) * QW], np.float32)
            out[b, qp * QW:(qp + 1) * QW, :] = acc.T
    return out


def kernel(queries, keys, values, valid_lens, W_q, W_k, W_v, W_o):
    order, kts, vls = plan(valid_lens)
    nc = build_nc(kts, vls)
    in_maps = make_in_maps(queries, keys, values, valid_lens,
                           W_q, W_k, W_v, W_o, order, kts)
    res = run_bass_kernel_spmd(nc, in_maps, list(range(N_CORES))).results
    return assemble(res, order)
